# revision 2
# baseline (speedup 1.0000x reference)
"""Trainium2 Bass kernel: dense transformer block (LN1-attn-LN2-FFN, causal, 16 heads).

Sharding (8 NeuronCores, SPMD one graph):
  - core j: token-parallel for LN/FFN/residual: owns tokens [512l, 512(l+1))
    of batch g, where g, l = divmod(j, 4)
  - attention head-parallel with cyclic head-batch assignment: core j computes
    head pair {2m, 2m+1}, m = (j + 4b) % 8, for EACH batch b over the full
    2048-token sequence.
  - comm: 2x half-token AllGather of feature-major LN1 output (pipelined with
    LN1 and QKV), AllToAll of normalized attention output (batch-0 shards
    shipped while batch 1 computes).
  - attention is kt-major (key-chunk stationary): both heads' score matmuls
    pack the PE array via row tiles (K=64 each, auto tile_position); exp in
    up-to-1024-col ACT calls, software-pipelined so scores of kt+1 issue
    while exp(kt) runs; causal masks multiplied on DVE; softmax denominator
    replicated across PSUM partitions 64:127 by ones-columns in the PV lhsT,
    normalized via one approx-reciprocal + fused DVE multiplies.
  - matmuls bf16 (f32 accumulate); residual stream f32.
  - LN gains/biases, 1/sqrt(dk), and bv are folded into weights host-side.
"""

import numpy as np
import ml_dtypes

import concourse.bass as bass
import concourse.tile as tile
from concourse import bacc, mybir
from concourse.bass_utils import run_bass_kernel_spmd

F32 = mybir.dt.float32
BF16 = mybir.dt.bfloat16
AF = mybir.ActivationFunctionType

D = 1024
DFF = 4096
B = 2
S = 2048
NCORES = 8
GRP = 4
TOK = 512
EPS = 1e-5

AGH = 128 * 8 * 256      # elems contributed per core per AG half (bf16)
A2A_N = NCORES * 128 * TOK


def build_nc():
    nc = bacc.Bacc("TRN2", target_bir_lowering=False, debug=False,
                   num_devices=NCORES)

    x_own = nc.dram_tensor("x_own", [TOK, D], F32, kind="ExternalInput").ap()
    wq = nc.dram_tensor("wq", [D, B, 128], BF16, kind="ExternalInput").ap()
    wk = nc.dram_tensor("wk", [D, B, 128], BF16, kind="ExternalInput").ap()
    wv = nc.dram_tensor("wv", [D, B, 128], BF16, kind="ExternalInput").ap()
    bq = nc.dram_tensor("bq", [B, 128], F32, kind="ExternalInput").ap()
    bk = nc.dram_tensor("bk", [B, 128], F32, kind="ExternalInput").ap()
    wo = nc.dram_tensor("wo", [D, D], BF16, kind="ExternalInput").ap()
    bo = nc.dram_tensor("bo", [D], F32, kind="ExternalInput").ap()
    w1 = nc.dram_tensor("w1", [D, DFF], BF16, kind="ExternalInput").ap()
    b1 = nc.dram_tensor("b1", [DFF], F32, kind="ExternalInput").ap()
    w2 = nc.dram_tensor("w2", [DFF, D], BF16, kind="ExternalInput").ap()
    b2 = nc.dram_tensor("b2", [D], F32, kind="ExternalInput").ap()
    me = nc.dram_tensor("me", [128, 256], BF16, kind="ExternalInput").ap()
    mo = nc.dram_tensor("mo", [128, 256], BF16, kind="ExternalInput").ap()
    id128 = nc.dram_tensor("id128", [128, 128], BF16, kind="ExternalInput").ap()
    out = nc.dram_tensor("out", [TOK, D], F32, kind="ExternalOutput").ap()

    rg = [list(range(NCORES))]

    with tile.TileContext(nc) as tc:
        with (
            tc.tile_pool(name="persist", bufs=1) as pp,
            tc.tile_pool(name="stage", bufs=4) as stg,
            tc.tile_pool(name="stats", bufs=4) as stp,
            tc.tile_pool(name="dram", bufs=1, space="DRAM") as dp,
        ):
            # ---- persistent SBUF (weights on ACT queue; x on SP queue) ----
            eps_sb = pp.tile([128, 1], F32, name="eps_sb")
            nc.vector.memset(eps_sb, EPS)
            id_sb = pp.tile([128, 128], BF16, name="id_sb")
            nc.scalar.dma_start(out=id_sb, in_=id128)
            me_sb = pp.tile([128, 256], BF16, name="me_sb")
            nc.scalar.dma_start(out=me_sb, in_=me)
            mo_sb = pp.tile([128, 256], BF16, name="mo_sb")
            nc.scalar.dma_start(out=mo_sb, in_=mo)
            bo_sb = pp.tile([128, 8], F32, name="bo_sb")
            nc.scalar.dma_start(out=bo_sb, in_=bo.rearrange("(k p) -> p k", p=128))
            b1_sb = pp.tile([128, 32], F32, name="b1_sb")
            nc.scalar.dma_start(out=b1_sb, in_=b1.rearrange("(k p) -> p k", p=128))
            b2_sb = pp.tile([128, 8], F32, name="b2_sb")
            nc.scalar.dma_start(out=b2_sb, in_=b2.rearrange("(k p) -> p k", p=128))

            x_tok = pp.tile([128, 4, D], F32, name="x_tok")
            x4 = x_own.rearrange("(t p) d -> t p d", p=128)
            for t in range(4):
                nc.sync.dma_start(out=x_tok[:, t, :], in_=x4[t])

            wq_sb = pp.tile([128, 8, B, 128], BF16, name="wq_sb")
            nc.scalar.dma_start(out=wq_sb,
                                in_=wq.rearrange("(k p) b m -> p k b m", p=128))
            wk_sb = pp.tile([128, 8, B, 128], BF16, name="wk_sb")
            nc.scalar.dma_start(out=wk_sb,
                                in_=wk.rearrange("(k p) b m -> p k b m", p=128))
            wv_sb = pp.tile([128, 8, B, 128], BF16, name="wv_sb")
            nc.scalar.dma_start(out=wv_sb,
                                in_=wv.rearrange("(k p) b m -> p k b m", p=128))
            bq_sb = pp.tile([128, B], F32, name="bq_sb")
            nc.scalar.dma_start(out=bq_sb, in_=bq.rearrange("b p -> p b"))
            bk_sb = pp.tile([128, B], F32, name="bk_sb")
            nc.scalar.dma_start(out=bk_sb, in_=bk.rearrange("b p -> p b"))

            qT = pp.tile([128, B, S], BF16, name="qT")
            kT = pp.tile([128, B, S], BF16, name="kT")
            vtok = pp.tile([128, 32, 2, 128], BF16, name="vtok")
            nc.vector.memset(vtok[:, :, :, 64:128], 1.0)
            attnT = pp.tile([128, B, S], BF16, name="attnT")
            lnT = pp.tile([128, 8, TOK], BF16, name="lnT")

            # ---- DRAM ----
            ag_in = [dp.tile([AGH], BF16, name=f"ag_in{h}") for h in range(2)]
            ag_out = [dp.tile([NCORES * AGH], BF16, name=f"ag_out{h}",
                              addr_space="Shared") for h in range(2)]
            a2a_in = dp.tile([A2A_N], BF16, name="a2a_in")
            a2a_out = dp.tile([A2A_N], BF16, name="a2a_out")

            # ---- LN helper (token-major stats; bf16 out) ----
            def layernorm_chunk(src, t, xout):
                # src [128, 4, 1024] f32; writes xout [128, 1024] bf16
                xin = src[:, t, :]
                xg_ = xin.rearrange("p (g d) -> p g d", g=2)
                st_ = stp.tile([128, 2, 6], F32, name="st_")
                for gs in range(2):
                    nc.vector.bn_stats(out=st_[:, gs, :], in_=xg_[:, gs, :])
                mv = stp.tile([128, 2], F32, name="mv")
                nc.vector.bn_aggr(out=mv, in_=st_)
                rstd = stp.tile([128, 1], F32, name="rstd")
                nc.scalar.activation(out=rstd, in_=mv[:, 1:2], func=AF.Sqrt,
                                     bias=eps_sb, scale=1.0)
                nc.vector.reciprocal(out=rstd, in_=rstd)
                nc.vector.tensor_scalar(out=xout, in0=xin, scalar1=mv[:, 0:1],
                                        scalar2=rstd,
                                        op0=mybir.AluOpType.subtract,
                                        op1=mybir.AluOpType.mult)

            with (
                tc.tile_pool(name="xgp", bufs=1) as xgp,
                tc.tile_pool(name="psT1", bufs=2, space="PSUM") as psT1,
                tc.tile_pool(name="psA", bufs=3, space="PSUM") as psA,
                tc.tile_pool(name="psV", bufs=3, space="PSUM") as psV,
            ):
                def ln_transpose(src, t):
                    # LN of token chunk t -> feature-major lnT[:, :, t*128:]
                    xln = stg.tile([128, D], BF16, name="xln")
                    layernorm_chunk(src, t, xln)
                    for fb in range(8):
                        pt = psT1.tile([128, 128], BF16, name="pt")
                        nc.tensor.transpose(pt, xln[:, fb * 128:(fb + 1) * 128],
                                            id_sb)
                        nc.vector.tensor_copy(
                            out=lnT[:, fb, t * 128:(t + 1) * 128], in_=pt)

                # ---- LN1 + staged AllGather (2 token-halves) ----
                # ag half h ships lnT[:, :, h*256:(h+1)*256] in (p, k, t) order
                for h in range(2):
                    for tt in range(2):
                        ln_transpose(x_tok, 2 * h + tt)
                    nc.sync.dma_start(
                        out=ag_in[h].rearrange("(p k t) -> p k t", p=128, k=8),
                        in_=lnT[:, :, h * 256:(h + 1) * 256])
                    nc.gpsimd.collective_compute(
                        "AllGather", mybir.AluOpType.bypass, replica_groups=rg,
                        ins=[ag_in[h].opt()], outs=[ag_out[h].opt()])

                # xg: [128 feat, src, half, kc, 256 tok] bf16
                xg = xgp.tile([128, NCORES, 2, 8, 256], BF16, name="xg")
                for h in range(2):
                    ago = ag_out[h].rearrange("(r p k t) -> r p k t",
                                              r=NCORES, p=128, k=8)
                    for r in range(NCORES):
                        eng = [nc.sync, nc.scalar][r % 2]
                        eng.dma_start(out=xg[:, r, h], in_=ago[r])

                # ---- QKV per (half, src): Q,K feature-major; V token-major ----
                for h in range(2):
                    for r in range(NCORES):
                        b = r // 4
                        c0 = (r % 4) * 512 + h * 256
                        for dst, wsb, bsb in ((qT, wq_sb, bq_sb),
                                              (kT, wk_sb, bk_sb)):
                            acc = psA.tile([128, 256], F32, name="acc")
                            for kc in range(8):
                                nc.tensor.matmul(acc, lhsT=wsb[:, kc, b, :],
                                                 rhs=xg[:, r, h, kc, :],
                                                 start=(kc == 0), stop=(kc == 7))
                            nc.vector.tensor_scalar_add(
                                out=dst[:, b, c0:c0 + 256], in0=acc,
                                scalar1=bsb[:, b:b + 1])
                        for tc_ in range(2):
                            accv = psV.tile([128, 128], F32, name="accv")
                            for kc in range(8):
                                nc.tensor.matmul(
                                    accv,
                                    lhsT=xg[:, r, h, kc,
                                            tc_ * 128:tc_ * 128 + 128],
                                    rhs=wv_sb[:, kc, b, :],
                                    start=(kc == 0), stop=(kc == 7))
                            ch = b * 16 + (r % 4) * 4 + h * 2 + tc_
                            nc.vector.tensor_copy(
                                out=vtok[:, ch, :, 0:64],
                                in_=accv.rearrange("p (hd c) -> p hd c", hd=2))

            # ---- attention: kt-major, 2-head row-packed scores ----
            with tc.tile_pool(name="ffp", bufs=1) as ffp:
                af_sb = ffp.tile([128, 8, TOK], BF16, name="af_sb")
                wo_sb = ffp.tile([128, 8, D], BF16, name="wo_sb")
                nc.scalar.dma_start(
                    out=wo_sb, in_=wo.rearrange("(k p) m -> p k m", p=128))
                h1T = ffp.tile([128, 32, TOK], BF16, name="h1T")

                with (
                    tc.tile_pool(name="ptp", bufs=4) as ptp,
                    tc.tile_pool(name="rp", bufs=6) as rp,
                    tc.tile_pool(name="psS", bufs=2, space="PSUM") as psS,
                    tc.tile_pool(name="psO", bufs=4, space="PSUM") as psO,
                ):
                    def emit_pv(b, q2lo_h, q2hi, Ot, kt, q2lo, pt_):
                        for q2 in range(q2lo, q2hi):
                            qr = (q2 - q2lo) * 256
                            for hh in range(2):
                                # start=True clears has_written bits for the
                                # WHOLE bank; two q2 share one bank, so only
                                # the bank-first q2 may clear. The sibling's
                                # first write lands on cleared bits and
                                # overwrites correctly.
                                nc.tensor.matmul(
                                    Ot[hh][:, q2 - q2lo_h, :],
                                    lhsT=vtok[:, b * 16 + kt, hh, :],
                                    rhs=pt_[:, hh, qr:qr + 256],
                                    start=(kt == 0
                                           and (q2 - q2lo_h) % 2 == 0),
                                    stop=(kt == 2 * q2 + 1),
                                    skip_group_check=True)

                    # quarter-rounds: (b, qq) covers q2 pair (2qq, 2qq+1)
                    for b in range(B):
                        for qq in range(4):
                            q2lo_h, q2hi = 2 * qq, 2 * qq + 2
                            nkt = 4 * qq + 4
                            # O (per head): [128, 2 q2, 256]; rows 64:127 get
                            # the denominator replicated via vtok's ones cols
                            Ot = [psO.tile([128, 2, 256], F32, name="O")
                                  for _ in range(2)]
                            pend = None
                            for kt in range(nkt):
                                q2lo = max(q2lo_h, kt // 2)
                                ncols = (q2hi - q2lo) * 256
                                sc = psS.tile([128, 2, 512], F32, name="sc")
                                for hh in range(2):
                                    hp = hh * 64
                                    nc.tensor.matmul(
                                        sc[:, hh, :ncols],
                                        lhsT=kT[hp:hp + 64, b,
                                                kt * 128:kt * 128 + 128],
                                        rhs=qT[hp:hp + 64, b,
                                               q2lo * 256:q2lo * 256 + ncols],
                                        start=True, stop=True)
                                pt_ = ptp.tile([128, 2, 512], BF16,
                                               name="pt_")
                                nc.scalar.activation(out=pt_[:, :, :ncols],
                                                     in_=sc[:, :, :ncols],
                                                     func=AF.Exp)
                                # causal mask on the diagonal 256-block
                                if kt // 2 >= q2lo_h:
                                    msk = me_sb if kt % 2 == 0 else mo_sb
                                    for hh in range(2):
                                        nc.vector.tensor_mul(
                                            out=pt_[:, hh, 0:256],
                                            in0=pt_[:, hh, 0:256], in1=msk)
                                # software pipeline: PV of previous kt issues
                                # after this kt's score matmuls
                                if pend is not None:
                                    emit_pv(b, q2lo_h, q2hi, Ot, *pend)
                                pend = (kt, q2lo, pt_)
                            emit_pv(b, q2lo_h, q2hi, Ot, *pend)

                            # normalize: denominator rows are broadcast by the
                            # matmul; copy to SBUF (approx-recip can't read
                            # PSUM), reciprocal, fused multiply per head
                            for hh in range(2):
                                hp = hh * 64
                                for q2 in range(q2lo_h, q2hi):
                                    O2 = Ot[hh][:, q2 - q2lo_h, :]
                                    den = rp.tile([64, 256], F32, name="den")
                                    nc.vector.tensor_copy(out=den,
                                                          in_=O2[64:128, :])
                                    rc = rp.tile([64, 256], F32, name="rc")
                                    nc.vector.reciprocal_approx_fast(
                                        out=rc, in_=den)
                                    nc.vector.tensor_mul(
                                        out=attnT[hp:hp + 64, b,
                                                  q2 * 256:q2 * 256 + 256],
                                        in0=O2[0:64, :], in1=rc)

                        # ship this batch's shards while the next computes
                        nc.sync.dma_start(
                            out=a2a_in.rearrange("(s p t) -> p s t", s=8,
                                                 p=128)[:, b * 4:(b + 1) * 4],
                            in_=attnT[:, b, :].rearrange("p (jj t) -> p jj t",
                                                         jj=4))

                nc.gpsimd.collective_compute(
                    "AllToAll", mybir.AluOpType.bypass, replica_groups=rg,
                    ins=[a2a_in.opt()], outs=[a2a_out.opt()])

                _af3 = a2a_out.rearrange("(i p t) -> i p t", i=8, p=128)
                for i in range(8):
                    [nc.gpsimd, nc.scalar][i % 2].dma_start(
                        out=af_sb[:, i, :], in_=_af3[i])

                # ---- wo projection + residual; LN2; FFN ----
                with (
                    tc.tile_pool(name="psW", bufs=3, space="PSUM") as psW,
                    tc.tile_pool(name="psT2", bufs=2, space="PSUM") as psT2,
                    tc.tile_pool(name="w2p", bufs=6) as w2p,
                ):
                    def resid_add(ybf, fb):
                        # ybf [128 feat, 512 tok] bf16 -> x_tok += y^T
                        for t in range(4):
                            pt = psT2.tile([128, 128], BF16, name="pt")
                            nc.tensor.transpose(
                                pt, ybf[:, t * 128:(t + 1) * 128], id_sb)
                            nc.vector.tensor_add(
                                out=x_tok[:, t, fb * 128:(fb + 1) * 128],
                                in0=x_tok[:, t, fb * 128:(fb + 1) * 128],
                                in1=pt)

                    for fb in range(8):
                        acc = psW.tile([128, TOK], F32, name="acc")
                        for kc in range(8):
                            nc.tensor.matmul(
                                acc, lhsT=wo_sb[:, kc, fb * 128:(fb + 1) * 128],
                                rhs=af_sb[:, kc, :],
                                start=(kc == 0), stop=(kc == 7))
                        yT = stg.tile([128, TOK], BF16, name="yT")
                        nc.vector.tensor_scalar_add(out=yT, in0=acc,
                                                    scalar1=bo_sb[:, fb:fb + 1])
                        resid_add(yT, fb)

                    # ---- LN2 -> lnT (PE transposes) ----
                    for t in range(4):
                        xln = stg.tile([128, D], BF16, name="xln")
                        layernorm_chunk(x_tok, t, xln)
                        for fb in range(8):
                            pt = psT2.tile([128, 128], BF16, name="pt")
                            nc.tensor.transpose(
                                pt, xln[:, fb * 128:(fb + 1) * 128], id_sb)
                            nc.vector.tensor_copy(
                                out=lnT[:, fb, t * 128:(t + 1) * 128], in_=pt)

                    # ---- FFN1 ----
                    with tc.tile_pool(name="w1p", bufs=6) as w1p:
                        for hbk in range(32):
                            w1t = w1p.tile([128, 8, 128], BF16, name="w1t")
                            nc.sync.dma_start(
                                out=w1t,
                                in_=w1[:, hbk * 128:(hbk + 1) * 128]
                                .rearrange("(k p) m -> p k m", p=128))
                            acc = psW.tile([128, TOK], F32, name="acc")
                            for kc in range(8):
                                nc.tensor.matmul(acc, lhsT=w1t[:, kc, :],
                                                 rhs=lnT[:, kc, :],
                                                 start=(kc == 0),
                                                 stop=(kc == 7))
                            nc.scalar.activation(out=h1T[:, hbk, :], in_=acc,
                                                 func=AF.Gelu,
                                                 bias=b1_sb[:, hbk:hbk + 1],
                                                 scale=1.0)

                    # ---- FFN2 + residual + streamed output ----
                    for fb in range(8):
                        acc = psW.tile([128, TOK], F32, name="acc")
                        for hg in range(4):
                            w2t = w2p.tile([128, 8, 128], BF16, name="w2t")
                            nc.sync.dma_start(
                                out=w2t,
                                in_=w2[hg * 1024:(hg + 1) * 1024,
                                       fb * 128:(fb + 1) * 128]
                                .rearrange("(k p) m -> p k m", p=128))
                            for kc in range(8):
                                nc.tensor.matmul(acc, lhsT=w2t[:, kc, :],
                                                 rhs=h1T[:, hg * 8 + kc, :],
                                                 start=(hg == 0 and kc == 0),
                                                 stop=(hg == 3 and kc == 7))
                        y2T = stg.tile([128, TOK], BF16, name="y2T")
                        nc.vector.tensor_scalar_add(out=y2T, in0=acc,
                                                    scalar1=b2_sb[:, fb:fb + 1])
                        resid_add(y2T, fb)
                        nc.gpsimd.dma_start(
                            out=out.rearrange("(t p) d -> p t d", p=128)
                            [:, :, fb * 128:(fb + 1) * 128],
                            in_=x_tok[:, :, fb * 128:(fb + 1) * 128])

    nc.compile()
    return nc


_NC_CACHE = {}


def _get_nc():
    if "nc" not in _NC_CACHE:
        _NC_CACHE["nc"] = build_nc()
    return _NC_CACHE["nc"]


def _prep_in_maps(x, ln1_g, ln1_b, wq, bq, wk, bk, wv, bv, wo, bo,
                  ln2_g, ln2_b, w1, b1, w2, b2):
    bf16 = ml_dtypes.bfloat16
    f32 = np.float32
    x = np.asarray(x, f32)
    DK = 64
    sc = 1.0 / np.sqrt(DK)
    ln1_g = np.asarray(ln1_g, f32)
    ln1_b = np.asarray(ln1_b, f32)
    ln2_g = np.asarray(ln2_g, f32)
    ln2_b = np.asarray(ln2_b, f32)
    wq = np.asarray(wq, f32)
    wk = np.asarray(wk, f32)
    wv = np.asarray(wv, f32)
    wo_np = np.asarray(wo, f32)
    w1 = np.asarray(w1, f32)
    w2 = np.asarray(w2, f32)

    wq_f = (ln1_g[:, None] * wq * sc).astype(bf16)
    bq_f = ((ln1_b @ wq + np.asarray(bq, f32)) * sc).astype(f32)
    wk_f = (ln1_g[:, None] * wk).astype(bf16)
    bk_f = (ln1_b @ wk + np.asarray(bk, f32)).astype(f32)
    wv_f = (ln1_g[:, None] * wv).astype(bf16)
    bv_f = (ln1_b @ wv + np.asarray(bv, f32)).astype(f32)
    bo_f = (np.asarray(bo, f32) + bv_f @ wo_np).astype(f32)
    wo_f = wo_np.astype(bf16)
    w1_f = (ln2_g[:, None] * w1).astype(bf16)
    b1_f = (ln2_b @ w1 + np.asarray(b1, f32)).astype(f32)
    w2_f = w2.astype(bf16)
    b2_f = np.asarray(b2, f32)

    tri = np.triu(np.ones((128, 128), f32))
    me_np = np.concatenate([tri, np.ones((128, 128), f32)], 1).astype(bf16)
    mo_np = np.concatenate([np.zeros((128, 128), f32), tri], 1).astype(bf16)
    id_np = np.eye(128, dtype=f32).astype(bf16)

    in_maps = []
    for core in range(NCORES):
        g, l = divmod(core, GRP)

        def hsel(b, j=core):
            m = (j + 4 * b) % 8
            return slice(m * 128, (m + 1) * 128)

        wo_perm = np.concatenate(
            [wo_f[((i + 4 * g) % 8) * 128:((i + 4 * g) % 8) * 128 + 128, :]
             for i in range(8)], axis=0)
        in_maps.append({
            "x_own": np.ascontiguousarray(x[g, l * TOK:(l + 1) * TOK, :]),
            "wq": np.ascontiguousarray(
                np.stack([wq_f[:, hsel(b)] for b in range(B)], axis=1)),
            "wk": np.ascontiguousarray(
                np.stack([wk_f[:, hsel(b)] for b in range(B)], axis=1)),
            "wv": np.ascontiguousarray(
                np.stack([wv_f[:, hsel(b)] for b in range(B)], axis=1)),
            "bq": np.ascontiguousarray(
                np.stack([bq_f[hsel(b)] for b in range(B)])),
            "bk": np.ascontiguousarray(
                np.stack([bk_f[hsel(b)] for b in range(B)])),
            "wo": np.ascontiguousarray(wo_perm), "bo": bo_f,
            "w1": w1_f, "b1": b1_f, "w2": w2_f, "b2": b2_f,
            "me": me_np, "mo": mo_np, "id128": id_np,
        })
    return in_maps


def kernel(**inputs):
    nc = _get_nc()
    in_maps = _prep_in_maps(**inputs)
    res = run_bass_kernel_spmd(nc, in_maps, core_ids=list(range(NCORES)))
    full = np.empty((B, S, D), np.float32)
    for core in range(NCORES):
        g, l = divmod(core, GRP)
        full[g, l * TOK:(l + 1) * TOK, :] = res.results[core]["out"]
    return full


# revision 3
# speedup vs baseline: 1.0412x; 1.0412x over previous
"""Trainium2 Bass kernel: dense transformer block (LN1-attn-LN2-FFN, causal, 16 heads).

Sharding (8 NeuronCores, SPMD one graph):
  - core j: token-parallel for LN/FFN/residual: owns tokens [512l, 512(l+1))
    of batch g, where g, l = divmod(j, 4)
  - attention head-parallel with cyclic head-batch assignment: core j computes
    head pair {2m, 2m+1}, m = (j + 4b) % 8, for EACH batch b over the full
    2048-token sequence.
  - comm: 2x half-token AllGather of feature-major LN1 output (pipelined with
    LN1 and QKV), AllToAll of normalized attention output (batch-0 shards
    shipped while batch 1 computes).
  - attention is kt-major (key-chunk stationary): both heads' score matmuls
    pack the PE array via row tiles (K=64 each, auto tile_position); exp in
    up-to-1024-col ACT calls, software-pipelined so scores of kt+1 issue
    while exp(kt) runs; causal masks multiplied on DVE; softmax denominator
    replicated across PSUM partitions 64:127 by ones-columns in the PV lhsT,
    normalized via one approx-reciprocal + fused DVE multiplies.
  - matmuls bf16 (f32 accumulate); residual stream f32.
  - LN gains/biases, 1/sqrt(dk), and bv are folded into weights host-side.
"""

import numpy as np
import ml_dtypes

import concourse.bass as bass
import concourse.tile as tile
from concourse import bacc, mybir
from concourse.bass_utils import run_bass_kernel_spmd

F32 = mybir.dt.float32
BF16 = mybir.dt.bfloat16
AF = mybir.ActivationFunctionType

D = 1024
DFF = 4096
B = 2
S = 2048
NCORES = 8
GRP = 4
TOK = 512
EPS = 1e-5

AGH = 128 * 8 * 256      # elems contributed per core per AG half (bf16)
A2A_N = NCORES * 128 * TOK


def build_nc():
    nc = bacc.Bacc("TRN2", target_bir_lowering=False, debug=False,
                   num_devices=NCORES)

    x_own = nc.dram_tensor("x_own", [TOK, D], F32, kind="ExternalInput").ap()
    wq = nc.dram_tensor("wq", [128, 8, B, 128], BF16, kind="ExternalInput").ap()
    wk = nc.dram_tensor("wk", [128, 8, B, 128], BF16, kind="ExternalInput").ap()
    wv = nc.dram_tensor("wv", [128, 8, B, 128], BF16, kind="ExternalInput").ap()
    bq = nc.dram_tensor("bq", [B, 128], F32, kind="ExternalInput").ap()
    bk = nc.dram_tensor("bk", [B, 128], F32, kind="ExternalInput").ap()
    wo = nc.dram_tensor("wo", [128, 8, D], BF16, kind="ExternalInput").ap()
    bo = nc.dram_tensor("bo", [D], F32, kind="ExternalInput").ap()
    w1 = nc.dram_tensor("w1", [128, 32, 8, 128], BF16, kind="ExternalInput").ap()
    b1 = nc.dram_tensor("b1", [DFF], F32, kind="ExternalInput").ap()
    w2 = nc.dram_tensor("w2", [128, 8, 4, 8, 128], BF16, kind="ExternalInput").ap()
    b2 = nc.dram_tensor("b2", [D], F32, kind="ExternalInput").ap()
    me = nc.dram_tensor("me", [128, 256], BF16, kind="ExternalInput").ap()
    mo = nc.dram_tensor("mo", [128, 256], BF16, kind="ExternalInput").ap()
    id128 = nc.dram_tensor("id128", [128, 128], BF16, kind="ExternalInput").ap()
    out = nc.dram_tensor("out", [TOK, D], F32, kind="ExternalOutput").ap()

    rg = [list(range(NCORES))]

    with tile.TileContext(nc) as tc:
        with (
            tc.tile_pool(name="persist", bufs=1) as pp,
            tc.tile_pool(name="stage", bufs=4) as stg,
            tc.tile_pool(name="stats", bufs=4) as stp,
            tc.tile_pool(name="dram", bufs=1, space="DRAM") as dp,
        ):
            # ---- persistent SBUF (weights on ACT queue; x on SP queue) ----
            eps_sb = pp.tile([128, 1], F32, name="eps_sb")
            nc.vector.memset(eps_sb, EPS)
            id_sb = pp.tile([128, 128], BF16, name="id_sb")
            nc.scalar.dma_start(out=id_sb, in_=id128)
            me_sb = pp.tile([128, 256], BF16, name="me_sb")
            nc.scalar.dma_start(out=me_sb, in_=me)
            mo_sb = pp.tile([128, 256], BF16, name="mo_sb")
            nc.scalar.dma_start(out=mo_sb, in_=mo)
            bo_sb = pp.tile([128, 8], F32, name="bo_sb")
            nc.scalar.dma_start(out=bo_sb, in_=bo.rearrange("(k p) -> p k", p=128))
            b1_sb = pp.tile([128, 32], F32, name="b1_sb")
            nc.scalar.dma_start(out=b1_sb, in_=b1.rearrange("(k p) -> p k", p=128))
            b2_sb = pp.tile([128, 8], F32, name="b2_sb")
            nc.scalar.dma_start(out=b2_sb, in_=b2.rearrange("(k p) -> p k", p=128))

            x_tok = pp.tile([128, 4, D], F32, name="x_tok")
            x4 = x_own.rearrange("(t p) d -> t p d", p=128)
            for t in range(4):
                nc.sync.dma_start(out=x_tok[:, t, :], in_=x4[t])

            wq_sb = pp.tile([128, 8, B, 128], BF16, name="wq_sb")
            nc.scalar.dma_start(out=wq_sb, in_=wq)
            wk_sb = pp.tile([128, 8, B, 128], BF16, name="wk_sb")
            nc.scalar.dma_start(out=wk_sb, in_=wk)
            wv_sb = pp.tile([128, 8, B, 128], BF16, name="wv_sb")
            nc.scalar.dma_start(out=wv_sb, in_=wv)
            bq_sb = pp.tile([128, B], F32, name="bq_sb")
            nc.scalar.dma_start(out=bq_sb, in_=bq.rearrange("b p -> p b"))
            bk_sb = pp.tile([128, B], F32, name="bk_sb")
            nc.scalar.dma_start(out=bk_sb, in_=bk.rearrange("b p -> p b"))

            qT = pp.tile([128, B, S], BF16, name="qT")
            kT = pp.tile([128, B, S], BF16, name="kT")
            vtok = pp.tile([128, 32, 2, 128], BF16, name="vtok")
            nc.vector.memset(vtok[:, :, :, 64:128], 1.0)
            attnT = pp.tile([128, B, S], BF16, name="attnT")
            lnT = pp.tile([128, 8, TOK], BF16, name="lnT")

            # ---- DRAM ----
            ag_in = [dp.tile([AGH], BF16, name=f"ag_in{h}") for h in range(2)]
            ag_out = [dp.tile([NCORES * AGH], BF16, name=f"ag_out{h}",
                              addr_space="Shared") for h in range(2)]
            a2a_in = dp.tile([A2A_N], BF16, name="a2a_in")
            a2a_out = dp.tile([A2A_N], BF16, name="a2a_out")

            # ---- LN helper (token-major stats; bf16 out) ----
            def layernorm_chunk(src, t, xout):
                # src [128, 4, 1024] f32; writes xout [128, 1024] bf16
                xin = src[:, t, :]
                xg_ = xin.rearrange("p (g d) -> p g d", g=2)
                st_ = stp.tile([128, 2, 6], F32, name="st_")
                for gs in range(2):
                    nc.vector.bn_stats(out=st_[:, gs, :], in_=xg_[:, gs, :])
                mv = stp.tile([128, 2], F32, name="mv")
                nc.vector.bn_aggr(out=mv, in_=st_)
                rstd = stp.tile([128, 1], F32, name="rstd")
                nc.scalar.activation(out=rstd, in_=mv[:, 1:2], func=AF.Sqrt,
                                     bias=eps_sb, scale=1.0)
                nc.vector.reciprocal(out=rstd, in_=rstd)
                nc.vector.tensor_scalar(out=xout, in0=xin, scalar1=mv[:, 0:1],
                                        scalar2=rstd,
                                        op0=mybir.AluOpType.subtract,
                                        op1=mybir.AluOpType.mult)

            with (
                tc.tile_pool(name="xgp", bufs=1) as xgp,
                tc.tile_pool(name="psT1", bufs=2, space="PSUM") as psT1,
                tc.tile_pool(name="psA", bufs=3, space="PSUM") as psA,
                tc.tile_pool(name="psV", bufs=3, space="PSUM") as psV,
            ):
                def ln_transpose(src, t):
                    # LN of token chunk t -> feature-major lnT[:, :, t*128:]
                    xln = stg.tile([128, D], BF16, name="xln")
                    layernorm_chunk(src, t, xln)
                    for fb in range(8):
                        pt = psT1.tile([128, 128], BF16, name="pt")
                        nc.tensor.transpose(pt, xln[:, fb * 128:(fb + 1) * 128],
                                            id_sb)
                        nc.vector.tensor_copy(
                            out=lnT[:, fb, t * 128:(t + 1) * 128], in_=pt)

                # ---- LN1 + staged AllGather (2 token-halves) ----
                # ag half h ships lnT[:, :, h*256:(h+1)*256] in (p, k, t) order
                for h in range(2):
                    for tt in range(2):
                        ln_transpose(x_tok, 2 * h + tt)
                    nc.sync.dma_start(
                        out=ag_in[h].rearrange("(p k t) -> p k t", p=128, k=8),
                        in_=lnT[:, :, h * 256:(h + 1) * 256])
                    nc.gpsimd.collective_compute(
                        "AllGather", mybir.AluOpType.bypass, replica_groups=rg,
                        ins=[ag_in[h].opt()], outs=[ag_out[h].opt()])

                # xg: [128 feat, src, half, kc, 256 tok] bf16
                xg = xgp.tile([128, NCORES, 2, 8, 256], BF16, name="xg")
                for h in range(2):
                    ago = ag_out[h].rearrange("(r p k t) -> r p k t",
                                              r=NCORES, p=128, k=8)
                    for r in range(NCORES):
                        eng = [nc.sync, nc.scalar][r % 2]
                        eng.dma_start(out=xg[:, r, h], in_=ago[r])

                # ---- QKV per (half, src): Q,K feature-major; V token-major ----
                for h in range(2):
                    for r in range(NCORES):
                        b = r // 4
                        c0 = (r % 4) * 512 + h * 256
                        for dst, wsb, bsb in ((qT, wq_sb, bq_sb),
                                              (kT, wk_sb, bk_sb)):
                            acc = psA.tile([128, 256], F32, name="acc")
                            for kc in range(8):
                                nc.tensor.matmul(acc, lhsT=wsb[:, kc, b, :],
                                                 rhs=xg[:, r, h, kc, :],
                                                 start=(kc == 0), stop=(kc == 7))
                            nc.vector.tensor_scalar_add(
                                out=dst[:, b, c0:c0 + 256], in0=acc,
                                scalar1=bsb[:, b:b + 1])
                        for tc_ in range(2):
                            accv = psV.tile([128, 128], F32, name="accv")
                            for kc in range(8):
                                nc.tensor.matmul(
                                    accv,
                                    lhsT=xg[:, r, h, kc,
                                            tc_ * 128:tc_ * 128 + 128],
                                    rhs=wv_sb[:, kc, b, :],
                                    start=(kc == 0), stop=(kc == 7))
                            ch = b * 16 + (r % 4) * 4 + h * 2 + tc_
                            nc.vector.tensor_copy(
                                out=vtok[:, ch, :, 0:64],
                                in_=accv.rearrange("p (hd c) -> p hd c", hd=2))

            # ---- attention: kt-major, 2-head row-packed scores ----
            with tc.tile_pool(name="ffp", bufs=1) as ffp:
                af_sb = ffp.tile([128, 8, TOK], BF16, name="af_sb")
                wo_sb = ffp.tile([128, 8, D], BF16, name="wo_sb")
                nc.scalar.dma_start(out=wo_sb, in_=wo)
                h1T = ffp.tile([128, 32, TOK], BF16, name="h1T")

                with (
                    tc.tile_pool(name="ptp", bufs=4) as ptp,
                    tc.tile_pool(name="rp", bufs=6) as rp,
                    tc.tile_pool(name="psS", bufs=2, space="PSUM") as psS,
                    tc.tile_pool(name="psO", bufs=4, space="PSUM") as psO,
                ):
                    def emit_pv(b, q2lo_h, q2hi, Ot, kt, q2lo, pt_):
                        for q2 in range(q2lo, q2hi):
                            qr = (q2 - q2lo) * 256
                            for hh in range(2):
                                # start=True clears has_written bits for the
                                # WHOLE bank; two q2 share one bank, so only
                                # the bank-first q2 may clear. The sibling's
                                # first write lands on cleared bits and
                                # overwrites correctly.
                                nc.tensor.matmul(
                                    Ot[hh][:, q2 - q2lo_h, :],
                                    lhsT=vtok[:, b * 16 + kt, hh, :],
                                    rhs=pt_[:, hh, qr:qr + 256],
                                    start=(kt == 0
                                           and (q2 - q2lo_h) % 2 == 0),
                                    stop=(kt == 2 * q2 + 1),
                                    skip_group_check=True)

                    # quarter-rounds: (b, qq) covers q2 pair (2qq, 2qq+1)
                    for b in range(B):
                        for qq in range(4):
                            q2lo_h, q2hi = 2 * qq, 2 * qq + 2
                            nkt = 4 * qq + 4
                            # O (per head): [128, 2 q2, 256]; rows 64:127 get
                            # the denominator replicated via vtok's ones cols
                            Ot = [psO.tile([128, 2, 256], F32, name="O")
                                  for _ in range(2)]
                            pend = None
                            for kt in range(nkt):
                                q2lo = max(q2lo_h, kt // 2)
                                ncols = (q2hi - q2lo) * 256
                                sc = psS.tile([128, 2, 512], F32, name="sc")
                                for hh in range(2):
                                    hp = hh * 64
                                    nc.tensor.matmul(
                                        sc[:, hh, :ncols],
                                        lhsT=kT[hp:hp + 64, b,
                                                kt * 128:kt * 128 + 128],
                                        rhs=qT[hp:hp + 64, b,
                                               q2lo * 256:q2lo * 256 + ncols],
                                        start=True, stop=True)
                                pt_ = ptp.tile([128, 2, 512], BF16,
                                               name="pt_")
                                nc.scalar.activation(out=pt_[:, :, :ncols],
                                                     in_=sc[:, :, :ncols],
                                                     func=AF.Exp)
                                # causal mask on the diagonal 256-block
                                if kt // 2 >= q2lo_h:
                                    msk = me_sb if kt % 2 == 0 else mo_sb
                                    for hh in range(2):
                                        nc.vector.tensor_mul(
                                            out=pt_[:, hh, 0:256],
                                            in0=pt_[:, hh, 0:256], in1=msk)
                                # software pipeline: PV of previous kt issues
                                # after this kt's score matmuls
                                if pend is not None:
                                    emit_pv(b, q2lo_h, q2hi, Ot, *pend)
                                pend = (kt, q2lo, pt_)
                            emit_pv(b, q2lo_h, q2hi, Ot, *pend)

                            # normalize: denominator rows are broadcast by the
                            # matmul; copy to SBUF (approx-recip can't read
                            # PSUM), reciprocal, fused multiply per head
                            for hh in range(2):
                                hp = hh * 64
                                for q2 in range(q2lo_h, q2hi):
                                    O2 = Ot[hh][:, q2 - q2lo_h, :]
                                    den = rp.tile([64, 256], F32, name="den")
                                    nc.vector.tensor_copy(out=den,
                                                          in_=O2[64:128, :])
                                    rc = rp.tile([64, 256], F32, name="rc")
                                    nc.vector.reciprocal_approx_fast(
                                        out=rc, in_=den)
                                    nc.vector.tensor_mul(
                                        out=attnT[hp:hp + 64, b,
                                                  q2 * 256:q2 * 256 + 256],
                                        in0=O2[0:64, :], in1=rc)

                        # ship this batch's shards while the next computes
                        nc.sync.dma_start(
                            out=a2a_in.rearrange("(s p t) -> p s t", s=8,
                                                 p=128)[:, b * 4:(b + 1) * 4],
                            in_=attnT[:, b, :].rearrange("p (jj t) -> p jj t",
                                                         jj=4))

                nc.gpsimd.collective_compute(
                    "AllToAll", mybir.AluOpType.bypass, replica_groups=rg,
                    ins=[a2a_in.opt()], outs=[a2a_out.opt()])

                _af3 = a2a_out.rearrange("(i p t) -> i p t", i=8, p=128)
                for i in range(8):
                    [nc.gpsimd, nc.scalar][i % 2].dma_start(
                        out=af_sb[:, i, :], in_=_af3[i])

                # ---- wo projection + residual; LN2; FFN ----
                with (
                    tc.tile_pool(name="psW", bufs=3, space="PSUM") as psW,
                    tc.tile_pool(name="psT2", bufs=2, space="PSUM") as psT2,
                    tc.tile_pool(name="w2p", bufs=6) as w2p,
                ):
                    def resid_add(ybf, fb):
                        # ybf [128 feat, 512 tok] bf16 -> x_tok += y^T
                        for t in range(4):
                            pt = psT2.tile([128, 128], BF16, name="pt")
                            nc.tensor.transpose(
                                pt, ybf[:, t * 128:(t + 1) * 128], id_sb)
                            nc.vector.tensor_add(
                                out=x_tok[:, t, fb * 128:(fb + 1) * 128],
                                in0=x_tok[:, t, fb * 128:(fb + 1) * 128],
                                in1=pt)

                    for fb in range(8):
                        acc = psW.tile([128, TOK], F32, name="acc")
                        for kc in range(8):
                            nc.tensor.matmul(
                                acc, lhsT=wo_sb[:, kc, fb * 128:(fb + 1) * 128],
                                rhs=af_sb[:, kc, :],
                                start=(kc == 0), stop=(kc == 7))
                        yT = stg.tile([128, TOK], BF16, name="yT")
                        nc.vector.tensor_scalar_add(out=yT, in0=acc,
                                                    scalar1=bo_sb[:, fb:fb + 1])
                        resid_add(yT, fb)

                    # ---- LN2 -> lnT (PE transposes) ----
                    for t in range(4):
                        xln = stg.tile([128, D], BF16, name="xln")
                        layernorm_chunk(x_tok, t, xln)
                        for fb in range(8):
                            pt = psT2.tile([128, 128], BF16, name="pt")
                            nc.tensor.transpose(
                                pt, xln[:, fb * 128:(fb + 1) * 128], id_sb)
                            nc.vector.tensor_copy(
                                out=lnT[:, fb, t * 128:(t + 1) * 128], in_=pt)

                    # ---- FFN1 ----
                    with tc.tile_pool(name="w1p", bufs=6) as w1p:
                        for hbk in range(32):
                            w1t = w1p.tile([128, 8, 128], BF16, name="w1t")
                            nc.sync.dma_start(out=w1t, in_=w1[:, hbk])
                            acc = psW.tile([128, TOK], F32, name="acc")
                            for kc in range(8):
                                nc.tensor.matmul(acc, lhsT=w1t[:, kc, :],
                                                 rhs=lnT[:, kc, :],
                                                 start=(kc == 0),
                                                 stop=(kc == 7))
                            nc.scalar.activation(out=h1T[:, hbk, :], in_=acc,
                                                 func=AF.Gelu,
                                                 bias=b1_sb[:, hbk:hbk + 1],
                                                 scale=1.0)

                    # ---- FFN2 + residual + streamed output ----
                    for fb in range(8):
                        acc = psW.tile([128, TOK], F32, name="acc")
                        for hg in range(4):
                            w2t = w2p.tile([128, 8, 128], BF16, name="w2t")
                            nc.sync.dma_start(out=w2t, in_=w2[:, fb, hg])
                            for kc in range(8):
                                nc.tensor.matmul(acc, lhsT=w2t[:, kc, :],
                                                 rhs=h1T[:, hg * 8 + kc, :],
                                                 start=(hg == 0 and kc == 0),
                                                 stop=(hg == 3 and kc == 7))
                        y2T = stg.tile([128, TOK], BF16, name="y2T")
                        nc.vector.tensor_scalar_add(out=y2T, in0=acc,
                                                    scalar1=b2_sb[:, fb:fb + 1])
                        resid_add(y2T, fb)
                        nc.gpsimd.dma_start(
                            out=out.rearrange("(t p) d -> p t d", p=128)
                            [:, :, fb * 128:(fb + 1) * 128],
                            in_=x_tok[:, :, fb * 128:(fb + 1) * 128])

    nc.compile()
    return nc


_NC_CACHE = {}


def _get_nc():
    if "nc" not in _NC_CACHE:
        _NC_CACHE["nc"] = build_nc()
    return _NC_CACHE["nc"]


def _prep_in_maps(x, ln1_g, ln1_b, wq, bq, wk, bk, wv, bv, wo, bo,
                  ln2_g, ln2_b, w1, b1, w2, b2):
    bf16 = ml_dtypes.bfloat16
    f32 = np.float32
    x = np.asarray(x, f32)
    DK = 64
    sc = 1.0 / np.sqrt(DK)
    ln1_g = np.asarray(ln1_g, f32)
    ln1_b = np.asarray(ln1_b, f32)
    ln2_g = np.asarray(ln2_g, f32)
    ln2_b = np.asarray(ln2_b, f32)
    wq = np.asarray(wq, f32)
    wk = np.asarray(wk, f32)
    wv = np.asarray(wv, f32)
    wo_np = np.asarray(wo, f32)
    w1 = np.asarray(w1, f32)
    w2 = np.asarray(w2, f32)

    wq_f = (ln1_g[:, None] * wq * sc).astype(bf16)
    bq_f = ((ln1_b @ wq + np.asarray(bq, f32)) * sc).astype(f32)
    wk_f = (ln1_g[:, None] * wk).astype(bf16)
    bk_f = (ln1_b @ wk + np.asarray(bk, f32)).astype(f32)
    wv_f = (ln1_g[:, None] * wv).astype(bf16)
    bv_f = (ln1_b @ wv + np.asarray(bv, f32)).astype(f32)
    bo_f = (np.asarray(bo, f32) + bv_f @ wo_np).astype(f32)
    wo_f = wo_np.astype(bf16)
    w1_f = (ln2_g[:, None] * w1).astype(bf16)
    b1_f = (ln2_b @ w1 + np.asarray(b1, f32)).astype(f32)
    w2_f = w2.astype(bf16)
    b2_f = np.asarray(b2, f32)

    tri = np.triu(np.ones((128, 128), f32))
    me_np = np.concatenate([tri, np.ones((128, 128), f32)], 1).astype(bf16)
    mo_np = np.concatenate([np.zeros((128, 128), f32), tri], 1).astype(bf16)
    id_np = np.eye(128, dtype=f32).astype(bf16)

    in_maps = []
    for core in range(NCORES):
        g, l = divmod(core, GRP)

        def hsel(b, j=core):
            m = (j + 4 * b) % 8
            return slice(m * 128, (m + 1) * 128)

        wo_perm = np.concatenate(
            [wo_f[((i + 4 * g) % 8) * 128:((i + 4 * g) % 8) * 128 + 128, :]
             for i in range(8)], axis=0)

        def pmaj(a):   # [1024, ...] -> [128, 8, ...] partition-major
            return np.ascontiguousarray(
                a.reshape(8, 128, *a.shape[1:]).transpose(
                    1, 0, *range(2, a.ndim + 1)))

        in_maps.append({
            "x_own": np.ascontiguousarray(x[g, l * TOK:(l + 1) * TOK, :]),
            "wq": pmaj(np.stack([wq_f[:, hsel(b)] for b in range(B)], axis=1)),
            "wk": pmaj(np.stack([wk_f[:, hsel(b)] for b in range(B)], axis=1)),
            "wv": pmaj(np.stack([wv_f[:, hsel(b)] for b in range(B)], axis=1)),
            "bq": np.ascontiguousarray(
                np.stack([bq_f[hsel(b)] for b in range(B)])),
            "bk": np.ascontiguousarray(
                np.stack([bk_f[hsel(b)] for b in range(B)])),
            "wo": pmaj(wo_perm), "bo": bo_f,
            "w1": np.ascontiguousarray(
                w1_f.reshape(8, 128, 32, 128).transpose(1, 2, 0, 3)),
            "b1": b1_f,
            "w2": np.ascontiguousarray(
                w2_f.reshape(4, 8, 128, 8, 128).transpose(2, 3, 0, 1, 4)),
            "b2": b2_f,
            "me": me_np, "mo": mo_np, "id128": id_np,
        })
    return in_maps


def kernel(**inputs):
    nc = _get_nc()
    in_maps = _prep_in_maps(**inputs)
    res = run_bass_kernel_spmd(nc, in_maps, core_ids=list(range(NCORES)))
    full = np.empty((B, S, D), np.float32)
    for core in range(NCORES):
        g, l = divmod(core, GRP)
        full[g, l * TOK:(l + 1) * TOK, :] = res.results[core]["out"]
    return full


# revision 4
# speedup vs baseline: 1.0510x; 1.0094x over previous
"""Trainium2 Bass kernel: dense transformer block (LN1-attn-LN2-FFN, causal, 16 heads).

Sharding (8 NeuronCores, SPMD one graph):
  - core j: token-parallel for LN/FFN/residual: owns tokens [512l, 512(l+1))
    of batch g, where g, l = divmod(j, 4)
  - attention head-parallel with cyclic head-batch assignment: core j computes
    head pair {2m, 2m+1}, m = (j + 4b) % 8, for EACH batch b over the full
    2048-token sequence.
  - comm: 2x half-token AllGather of feature-major LN1 output (pipelined with
    LN1 and QKV), AllToAll of normalized attention output (batch-0 shards
    shipped while batch 1 computes).
  - attention is kt-major (key-chunk stationary): both heads' score matmuls
    pack the PE array via row tiles (K=64 each, auto tile_position); exp in
    up-to-1024-col ACT calls, software-pipelined so scores of kt+1 issue
    while exp(kt) runs; causal masks multiplied on DVE; softmax denominator
    replicated across PSUM partitions 64:127 by ones-columns in the PV lhsT,
    normalized via one approx-reciprocal + fused DVE multiplies.
  - matmuls bf16 (f32 accumulate); residual stream f32.
  - LN gains/biases, 1/sqrt(dk), and bv are folded into weights host-side.
"""

import numpy as np
import ml_dtypes

import concourse.bass as bass
import concourse.tile as tile
from concourse import bacc, mybir
from concourse.bass_utils import run_bass_kernel_spmd

F32 = mybir.dt.float32
BF16 = mybir.dt.bfloat16
AF = mybir.ActivationFunctionType

D = 1024
DFF = 4096
B = 2
S = 2048
NCORES = 8
GRP = 4
TOK = 512
EPS = 1e-5

AGH = 128 * 8 * 256      # elems contributed per core per AG half (bf16)
A2A_N = NCORES * 128 * TOK


def build_nc():
    nc = bacc.Bacc("TRN2", target_bir_lowering=False, debug=False,
                   num_devices=NCORES)

    x_own = nc.dram_tensor("x_own", [TOK, D], F32, kind="ExternalInput").ap()
    wq = nc.dram_tensor("wq", [128, 8, B, 128], mybir.dt.float8e4,
                        kind="ExternalInput").ap()
    wk = nc.dram_tensor("wk", [128, 8, B, 128], mybir.dt.float8e4,
                        kind="ExternalInput").ap()
    wv = nc.dram_tensor("wv", [128, 8, B, 128], mybir.dt.float8e4,
                        kind="ExternalInput").ap()
    bq = nc.dram_tensor("bq", [B, 128], F32, kind="ExternalInput").ap()
    bk = nc.dram_tensor("bk", [B, 128], F32, kind="ExternalInput").ap()
    wo = nc.dram_tensor("wo", [128, 8, D], BF16, kind="ExternalInput").ap()
    bo = nc.dram_tensor("bo", [D], F32, kind="ExternalInput").ap()
    w1 = nc.dram_tensor("w1", [128, 32, 8, 128], BF16, kind="ExternalInput").ap()
    b1 = nc.dram_tensor("b1", [DFF], F32, kind="ExternalInput").ap()
    w2 = nc.dram_tensor("w2", [128, 8, 4, 8, 128], BF16, kind="ExternalInput").ap()
    b2 = nc.dram_tensor("b2", [D], F32, kind="ExternalInput").ap()
    me = nc.dram_tensor("me", [128, 256], BF16, kind="ExternalInput").ap()
    mo = nc.dram_tensor("mo", [128, 256], BF16, kind="ExternalInput").ap()
    id128 = nc.dram_tensor("id128", [128, 128], BF16, kind="ExternalInput").ap()
    out = nc.dram_tensor("out", [TOK, D], F32, kind="ExternalOutput").ap()

    rg = [list(range(NCORES))]

    with tile.TileContext(nc) as tc:
        with (
            tc.tile_pool(name="persist", bufs=1) as pp,
            tc.tile_pool(name="stage", bufs=4) as stg,
            tc.tile_pool(name="stats", bufs=4) as stp,
            tc.tile_pool(name="dram", bufs=1, space="DRAM") as dp,
        ):
            # ---- persistent SBUF (weights on ACT queue; x on SP queue) ----
            eps_sb = pp.tile([128, 1], F32, name="eps_sb")
            nc.vector.memset(eps_sb, EPS)
            id_sb = pp.tile([128, 128], BF16, name="id_sb")
            nc.scalar.dma_start(out=id_sb, in_=id128)
            me_sb = pp.tile([128, 256], BF16, name="me_sb")
            nc.scalar.dma_start(out=me_sb, in_=me)
            mo_sb = pp.tile([128, 256], BF16, name="mo_sb")
            nc.scalar.dma_start(out=mo_sb, in_=mo)
            bo_sb = pp.tile([128, 8], F32, name="bo_sb")
            nc.scalar.dma_start(out=bo_sb, in_=bo.rearrange("(k p) -> p k", p=128))
            b1_sb = pp.tile([128, 32], F32, name="b1_sb")
            nc.scalar.dma_start(out=b1_sb, in_=b1.rearrange("(k p) -> p k", p=128))
            b2_sb = pp.tile([128, 8], F32, name="b2_sb")
            nc.scalar.dma_start(out=b2_sb, in_=b2.rearrange("(k p) -> p k", p=128))

            x_tok = pp.tile([128, 4, D], F32, name="x_tok")
            x4 = x_own.rearrange("(t p) d -> t p d", p=128)
            for t in range(4):
                nc.sync.dma_start(out=x_tok[:, t, :], in_=x4[t])

            wq_sb = pp.tile([128, 8, B, 128], mybir.dt.float8e4,
                           name="wq_sb")
            nc.scalar.dma_start(out=wq_sb, in_=wq)
            wk_sb = pp.tile([128, 8, B, 128], mybir.dt.float8e4,
                           name="wk_sb")
            nc.scalar.dma_start(out=wk_sb, in_=wk)
            wv_sb = pp.tile([128, 8, B, 128], mybir.dt.float8e4,
                           name="wv_sb")
            nc.scalar.dma_start(out=wv_sb, in_=wv)
            bq_sb = pp.tile([128, B], F32, name="bq_sb")
            nc.scalar.dma_start(out=bq_sb, in_=bq.rearrange("b p -> p b"))
            bk_sb = pp.tile([128, B], F32, name="bk_sb")
            nc.scalar.dma_start(out=bk_sb, in_=bk.rearrange("b p -> p b"))

            qT = pp.tile([128, B, S], BF16, name="qT")
            kT = pp.tile([128, B, S], BF16, name="kT")
            vtok = pp.tile([128, 32, 2, 128], BF16, name="vtok")
            nc.vector.memset(vtok[:, :, :, 64:128], 1.0)
            attnT = pp.tile([128, B, S], BF16, name="attnT")
            lnT = pp.tile([128, 8, TOK], BF16, name="lnT")
            lnT1 = pp.tile([128, 8, TOK], mybir.dt.float8e4, name="lnT1")

            # ---- DRAM ----
            ag_in = [dp.tile([AGH], mybir.dt.float8e4, name=f"ag_in{h}")
                     for h in range(2)]
            ag_out = [dp.tile([NCORES * AGH], mybir.dt.float8e4,
                              name=f"ag_out{h}", addr_space="Shared")
                      for h in range(2)]
            a2a_in = dp.tile([A2A_N], BF16, name="a2a_in")
            a2a_out = dp.tile([A2A_N], BF16, name="a2a_out")

            # ---- LN helper (token-major stats; bf16 out) ----
            def layernorm_chunk(src, t, xout):
                # src [128, 4, 1024] f32; writes xout [128, 1024] bf16
                xin = src[:, t, :]
                xg_ = xin.rearrange("p (g d) -> p g d", g=2)
                st_ = stp.tile([128, 2, 6], F32, name="st_")
                for gs in range(2):
                    nc.vector.bn_stats(out=st_[:, gs, :], in_=xg_[:, gs, :])
                mv = stp.tile([128, 2], F32, name="mv")
                nc.vector.bn_aggr(out=mv, in_=st_)
                rstd = stp.tile([128, 1], F32, name="rstd")
                nc.scalar.activation(out=rstd, in_=mv[:, 1:2], func=AF.Sqrt,
                                     bias=eps_sb, scale=1.0)
                nc.vector.reciprocal(out=rstd, in_=rstd)
                nc.vector.tensor_scalar(out=xout, in0=xin, scalar1=mv[:, 0:1],
                                        scalar2=rstd,
                                        op0=mybir.AluOpType.subtract,
                                        op1=mybir.AluOpType.mult)

            with (
                tc.tile_pool(name="xgp", bufs=1) as xgp,
                tc.tile_pool(name="psT1", bufs=2, space="PSUM") as psT1,
                tc.tile_pool(name="psA", bufs=3, space="PSUM") as psA,
                tc.tile_pool(name="psV", bufs=3, space="PSUM") as psV,
            ):
                def ln_transpose(src, t):
                    # LN of token chunk t -> feature-major lnT1 (fp8 cast in
                    # the PSUM->SBUF copy)
                    xln = stg.tile([128, D], BF16, name="xln")
                    layernorm_chunk(src, t, xln)
                    for fb in range(8):
                        pt = psT1.tile([128, 128], BF16, name="pt")
                        nc.tensor.transpose(pt, xln[:, fb * 128:(fb + 1) * 128],
                                            id_sb)
                        nc.vector.tensor_copy(
                            out=lnT1[:, fb, t * 128:(t + 1) * 128], in_=pt)

                # ---- LN1 + staged AllGather (2 token-halves, fp8) ----
                for h in range(2):
                    for tt in range(2):
                        ln_transpose(x_tok, 2 * h + tt)
                    nc.sync.dma_start(
                        out=ag_in[h].rearrange("(p k t) -> p k t", p=128, k=8),
                        in_=lnT1[:, :, h * 256:(h + 1) * 256])
                    nc.gpsimd.collective_compute(
                        "AllGather", mybir.AluOpType.bypass, replica_groups=rg,
                        ins=[ag_in[h].opt()], outs=[ag_out[h].opt()])

                # xg: [128 feat, src, half, kc, 256 tok] bf16
                xg = xgp.tile([128, NCORES, 2, 8, 256], mybir.dt.float8e4,
                              name="xg")
                for h in range(2):
                    ago = ag_out[h].rearrange("(r p k t) -> r p k t",
                                              r=NCORES, p=128, k=8)
                    for r in range(NCORES):
                        eng = [nc.sync, nc.scalar][r % 2]
                        eng.dma_start(out=xg[:, r, h], in_=ago[r])

                # ---- QKV per (half, src): Q,K feature-major; V token-major ----
                for h in range(2):
                    for r in range(NCORES):
                        b = r // 4
                        c0 = (r % 4) * 512 + h * 256
                        for dst, wsb, bsb in ((qT, wq_sb, bq_sb),
                                              (kT, wk_sb, bk_sb)):
                            acc = psA.tile([128, 256], F32, name="acc")
                            for kc in range(8):
                                nc.tensor.matmul(acc, lhsT=wsb[:, kc, b, :],
                                                 rhs=xg[:, r, h, kc, :],
                                                 start=(kc == 0), stop=(kc == 7))
                            nc.vector.tensor_scalar(
                                out=dst[:, b, c0:c0 + 256], in0=acc,
                                scalar1=1.0 / 64.0,
                                scalar2=bsb[:, b:b + 1],
                                op0=mybir.AluOpType.mult,
                                op1=mybir.AluOpType.add)
                        for tc_ in range(2):
                            accv = psV.tile([128, 128], F32, name="accv")
                            for kc in range(8):
                                nc.tensor.matmul(
                                    accv,
                                    lhsT=xg[:, r, h, kc,
                                            tc_ * 128:tc_ * 128 + 128],
                                    rhs=wv_sb[:, kc, b, :],
                                    start=(kc == 0), stop=(kc == 7))
                            ch = b * 16 + (r % 4) * 4 + h * 2 + tc_
                            nc.vector.tensor_scalar_mul(
                                out=vtok[:, ch, :, 0:64],
                                in0=accv.rearrange("p (hd c) -> p hd c", hd=2),
                                scalar1=1.0 / 64.0)

            # ---- attention: kt-major, 2-head row-packed scores ----
            with tc.tile_pool(name="ffp", bufs=1) as ffp:
                af_sb = ffp.tile([128, 8, TOK], BF16, name="af_sb")
                wo_sb = ffp.tile([128, 8, D], BF16, name="wo_sb")
                nc.scalar.dma_start(out=wo_sb, in_=wo)
                h1T = ffp.tile([128, 32, TOK], BF16, name="h1T")

                with (
                    tc.tile_pool(name="ptp", bufs=4) as ptp,
                    tc.tile_pool(name="rp", bufs=6) as rp,
                    tc.tile_pool(name="psS", bufs=2, space="PSUM") as psS,
                    tc.tile_pool(name="psO", bufs=4, space="PSUM") as psO,
                ):
                    def emit_pv(b, q2lo_h, q2hi, Ot, kt, q2lo, pt_):
                        for q2 in range(q2lo, q2hi):
                            qr = (q2 - q2lo) * 256
                            for hh in range(2):
                                # start=True clears has_written bits for the
                                # WHOLE bank; two q2 share one bank, so only
                                # the bank-first q2 may clear. The sibling's
                                # first write lands on cleared bits and
                                # overwrites correctly.
                                nc.tensor.matmul(
                                    Ot[hh][:, q2 - q2lo_h, :],
                                    lhsT=vtok[:, b * 16 + kt, hh, :],
                                    rhs=pt_[:, hh, qr:qr + 256],
                                    start=(kt == 0
                                           and (q2 - q2lo_h) % 2 == 0),
                                    stop=(kt == 2 * q2 + 1),
                                    skip_group_check=True)

                    # quarter-rounds: (b, qq) covers q2 pair (2qq, 2qq+1)
                    for b in range(B):
                        for qq in range(4):
                            q2lo_h, q2hi = 2 * qq, 2 * qq + 2
                            nkt = 4 * qq + 4
                            # O (per head): [128, 2 q2, 256]; rows 64:127 get
                            # the denominator replicated via vtok's ones cols
                            Ot = [psO.tile([128, 2, 256], F32, name="O")
                                  for _ in range(2)]
                            pend = None
                            for kt in range(nkt):
                                q2lo = max(q2lo_h, kt // 2)
                                ncols = (q2hi - q2lo) * 256
                                sc = psS.tile([128, 2, 512], F32, name="sc")
                                for hh in range(2):
                                    hp = hh * 64
                                    nc.tensor.matmul(
                                        sc[:, hh, :ncols],
                                        lhsT=kT[hp:hp + 64, b,
                                                kt * 128:kt * 128 + 128],
                                        rhs=qT[hp:hp + 64, b,
                                               q2lo * 256:q2lo * 256 + ncols],
                                        start=True, stop=True)
                                pt_ = ptp.tile([128, 2, 512], BF16,
                                               name="pt_")
                                nc.scalar.activation(out=pt_[:, :, :ncols],
                                                     in_=sc[:, :, :ncols],
                                                     func=AF.Exp)
                                # causal mask on the diagonal 256-block
                                if kt // 2 >= q2lo_h:
                                    msk = me_sb if kt % 2 == 0 else mo_sb
                                    for hh in range(2):
                                        nc.vector.tensor_mul(
                                            out=pt_[:, hh, 0:256],
                                            in0=pt_[:, hh, 0:256], in1=msk)
                                # software pipeline: PV of previous kt issues
                                # after this kt's score matmuls
                                if pend is not None:
                                    emit_pv(b, q2lo_h, q2hi, Ot, *pend)
                                pend = (kt, q2lo, pt_)
                            emit_pv(b, q2lo_h, q2hi, Ot, *pend)

                            # normalize: denominator rows are broadcast by the
                            # matmul; copy to SBUF (approx-recip can't read
                            # PSUM), reciprocal, fused multiply per head
                            for hh in range(2):
                                hp = hh * 64
                                for q2 in range(q2lo_h, q2hi):
                                    O2 = Ot[hh][:, q2 - q2lo_h, :]
                                    den = rp.tile([64, 256], F32, name="den")
                                    nc.vector.tensor_copy(out=den,
                                                          in_=O2[64:128, :])
                                    rc = rp.tile([64, 256], F32, name="rc")
                                    nc.vector.reciprocal_approx_fast(
                                        out=rc, in_=den)
                                    nc.vector.tensor_mul(
                                        out=attnT[hp:hp + 64, b,
                                                  q2 * 256:q2 * 256 + 256],
                                        in0=O2[0:64, :], in1=rc)

                        # ship this batch's shards while the next computes
                        nc.sync.dma_start(
                            out=a2a_in.rearrange("(s p t) -> p s t", s=8,
                                                 p=128)[:, b * 4:(b + 1) * 4],
                            in_=attnT[:, b, :].rearrange("p (jj t) -> p jj t",
                                                         jj=4))

                nc.gpsimd.collective_compute(
                    "AllToAll", mybir.AluOpType.bypass, replica_groups=rg,
                    ins=[a2a_in.opt()], outs=[a2a_out.opt()])

                _af3 = a2a_out.rearrange("(i p t) -> i p t", i=8, p=128)
                for i in range(8):
                    [nc.gpsimd, nc.scalar][i % 2].dma_start(
                        out=af_sb[:, i, :], in_=_af3[i])

                # ---- wo projection + residual; LN2; FFN ----
                with (
                    tc.tile_pool(name="psW", bufs=3, space="PSUM") as psW,
                    tc.tile_pool(name="psT2", bufs=2, space="PSUM") as psT2,
                    tc.tile_pool(name="w2p", bufs=6) as w2p,
                ):
                    def resid_add(ybf, fb):
                        # ybf [128 feat, 512 tok] bf16 -> x_tok += y^T
                        for t in range(4):
                            pt = psT2.tile([128, 128], BF16, name="pt")
                            nc.tensor.transpose(
                                pt, ybf[:, t * 128:(t + 1) * 128], id_sb)
                            nc.vector.tensor_add(
                                out=x_tok[:, t, fb * 128:(fb + 1) * 128],
                                in0=x_tok[:, t, fb * 128:(fb + 1) * 128],
                                in1=pt)

                    for fb in range(8):
                        acc = psW.tile([128, TOK], F32, name="acc")
                        for kc in range(8):
                            nc.tensor.matmul(
                                acc, lhsT=wo_sb[:, kc, fb * 128:(fb + 1) * 128],
                                rhs=af_sb[:, kc, :],
                                start=(kc == 0), stop=(kc == 7))
                        yT = stg.tile([128, TOK], BF16, name="yT")
                        nc.vector.tensor_scalar_add(out=yT, in0=acc,
                                                    scalar1=bo_sb[:, fb:fb + 1])
                        resid_add(yT, fb)

                    # ---- LN2 -> lnT (PE transposes) ----
                    for t in range(4):
                        xln = stg.tile([128, D], BF16, name="xln")
                        layernorm_chunk(x_tok, t, xln)
                        for fb in range(8):
                            pt = psT2.tile([128, 128], BF16, name="pt")
                            nc.tensor.transpose(
                                pt, xln[:, fb * 128:(fb + 1) * 128], id_sb)
                            nc.vector.tensor_copy(
                                out=lnT[:, fb, t * 128:(t + 1) * 128], in_=pt)

                    # ---- FFN1 ----
                    with tc.tile_pool(name="w1p", bufs=6) as w1p:
                        for hbk in range(32):
                            w1t = w1p.tile([128, 8, 128], BF16, name="w1t")
                            nc.sync.dma_start(out=w1t, in_=w1[:, hbk])
                            acc = psW.tile([128, TOK], F32, name="acc")
                            for kc in range(8):
                                nc.tensor.matmul(acc, lhsT=w1t[:, kc, :],
                                                 rhs=lnT[:, kc, :],
                                                 start=(kc == 0),
                                                 stop=(kc == 7))
                            nc.scalar.activation(out=h1T[:, hbk, :], in_=acc,
                                                 func=AF.Gelu,
                                                 bias=b1_sb[:, hbk:hbk + 1],
                                                 scale=1.0)

                    # ---- FFN2 + residual + streamed output ----
                    for fb in range(8):
                        acc = psW.tile([128, TOK], F32, name="acc")
                        for hg in range(4):
                            w2t = w2p.tile([128, 8, 128], BF16, name="w2t")
                            nc.sync.dma_start(out=w2t, in_=w2[:, fb, hg])
                            for kc in range(8):
                                nc.tensor.matmul(acc, lhsT=w2t[:, kc, :],
                                                 rhs=h1T[:, hg * 8 + kc, :],
                                                 start=(hg == 0 and kc == 0),
                                                 stop=(hg == 3 and kc == 7))
                        y2T = stg.tile([128, TOK], BF16, name="y2T")
                        nc.vector.tensor_scalar_add(out=y2T, in0=acc,
                                                    scalar1=b2_sb[:, fb:fb + 1])
                        resid_add(y2T, fb)
                        ov = out.rearrange("(t p) d -> p t d", p=128)
                        for t in range(4):
                            [nc.gpsimd, nc.scalar][t % 2].dma_start(
                                out=ov[:, t, fb * 128:(fb + 1) * 128],
                                in_=x_tok[:, t, fb * 128:(fb + 1) * 128])

    nc.compile()
    return nc


_NC_CACHE = {}


def _get_nc():
    if "nc" not in _NC_CACHE:
        _NC_CACHE["nc"] = build_nc()
    return _NC_CACHE["nc"]


def _prep_in_maps(x, ln1_g, ln1_b, wq, bq, wk, bk, wv, bv, wo, bo,
                  ln2_g, ln2_b, w1, b1, w2, b2):
    bf16 = ml_dtypes.bfloat16
    f32 = np.float32
    x = np.asarray(x, f32)
    DK = 64
    sc = 1.0 / np.sqrt(DK)
    ln1_g = np.asarray(ln1_g, f32)
    ln1_b = np.asarray(ln1_b, f32)
    ln2_g = np.asarray(ln2_g, f32)
    ln2_b = np.asarray(ln2_b, f32)
    wq = np.asarray(wq, f32)
    wk = np.asarray(wk, f32)
    wv = np.asarray(wv, f32)
    wo_np = np.asarray(wo, f32)
    w1 = np.asarray(w1, f32)
    w2 = np.asarray(w2, f32)

    f8 = ml_dtypes.float8_e4m3
    wq_f = (ln1_g[:, None] * wq * sc * 64.0).astype(f8)
    bq_f = ((ln1_b @ wq + np.asarray(bq, f32)) * sc).astype(f32)
    wk_f = (ln1_g[:, None] * wk * 64.0).astype(f8)
    bk_f = (ln1_b @ wk + np.asarray(bk, f32)).astype(f32)
    wv_f = (ln1_g[:, None] * wv * 64.0).astype(f8)
    bv_f = (ln1_b @ wv + np.asarray(bv, f32)).astype(f32)
    bo_f = (np.asarray(bo, f32) + bv_f @ wo_np).astype(f32)
    wo_f = wo_np.astype(bf16)
    w1_f = (ln2_g[:, None] * w1).astype(bf16)
    b1_f = (ln2_b @ w1 + np.asarray(b1, f32)).astype(f32)
    w2_f = w2.astype(bf16)
    b2_f = np.asarray(b2, f32)

    tri = np.triu(np.ones((128, 128), f32))
    me_np = np.concatenate([tri, np.ones((128, 128), f32)], 1).astype(bf16)
    mo_np = np.concatenate([np.zeros((128, 128), f32), tri], 1).astype(bf16)
    id_np = np.eye(128, dtype=f32).astype(bf16)

    in_maps = []
    for core in range(NCORES):
        g, l = divmod(core, GRP)

        def hsel(b, j=core):
            m = (j + 4 * b) % 8
            return slice(m * 128, (m + 1) * 128)

        wo_perm = np.concatenate(
            [wo_f[((i + 4 * g) % 8) * 128:((i + 4 * g) % 8) * 128 + 128, :]
             for i in range(8)], axis=0)

        def pmaj(a):   # [1024, ...] -> [128, 8, ...] partition-major
            return np.ascontiguousarray(
                a.reshape(8, 128, *a.shape[1:]).transpose(
                    1, 0, *range(2, a.ndim + 1)))

        in_maps.append({
            "x_own": np.ascontiguousarray(x[g, l * TOK:(l + 1) * TOK, :]),
            "wq": pmaj(np.stack([wq_f[:, hsel(b)] for b in range(B)], axis=1)),
            "wk": pmaj(np.stack([wk_f[:, hsel(b)] for b in range(B)], axis=1)),
            "wv": pmaj(np.stack([wv_f[:, hsel(b)] for b in range(B)], axis=1)),
            "bq": np.ascontiguousarray(
                np.stack([bq_f[hsel(b)] for b in range(B)])),
            "bk": np.ascontiguousarray(
                np.stack([bk_f[hsel(b)] for b in range(B)])),
            "wo": pmaj(wo_perm), "bo": bo_f,
            "w1": np.ascontiguousarray(
                w1_f.reshape(8, 128, 32, 128).transpose(1, 2, 0, 3)),
            "b1": b1_f,
            "w2": np.ascontiguousarray(
                w2_f.reshape(4, 8, 128, 8, 128).transpose(2, 3, 0, 1, 4)),
            "b2": b2_f,
            "me": me_np, "mo": mo_np, "id128": id_np,
        })
    return in_maps


def kernel(**inputs):
    nc = _get_nc()
    in_maps = _prep_in_maps(**inputs)
    res = run_bass_kernel_spmd(nc, in_maps, core_ids=list(range(NCORES)))
    full = np.empty((B, S, D), np.float32)
    for core in range(NCORES):
        g, l = divmod(core, GRP)
        full[g, l * TOK:(l + 1) * TOK, :] = res.results[core]["out"]
    return full


# revision 5
# speedup vs baseline: 1.0568x; 1.0055x over previous
"""Trainium2 Bass kernel: dense transformer block (LN1-attn-LN2-FFN, causal, 16 heads).

Sharding (8 NeuronCores, SPMD one graph):
  - core j: token-parallel for LN/FFN/residual: owns tokens [512l, 512(l+1))
    of batch g, where g, l = divmod(j, 4)
  - attention head-parallel with cyclic head-batch assignment: core j computes
    head pair {2m, 2m+1}, m = (j + 4b) % 8, for EACH batch b over the full
    2048-token sequence.
  - comm: 2x half-token AllGather of feature-major LN1 output (pipelined with
    LN1 and QKV), AllToAll of normalized attention output (batch-0 shards
    shipped while batch 1 computes).
  - attention is kt-major (key-chunk stationary): both heads' score matmuls
    pack the PE array via row tiles (K=64 each, auto tile_position); exp in
    up-to-1024-col ACT calls, software-pipelined so scores of kt+1 issue
    while exp(kt) runs; causal masks multiplied on DVE; softmax denominator
    replicated across PSUM partitions 64:127 by ones-columns in the PV lhsT,
    normalized via one approx-reciprocal + fused DVE multiplies.
  - matmuls bf16 (f32 accumulate); residual stream f32.
  - LN gains/biases, 1/sqrt(dk), and bv are folded into weights host-side.
"""

import numpy as np
import ml_dtypes

import concourse.bass as bass
import concourse.tile as tile
from concourse import bacc, mybir
from concourse.bass_utils import run_bass_kernel_spmd

F32 = mybir.dt.float32
BF16 = mybir.dt.bfloat16
AF = mybir.ActivationFunctionType

D = 1024
DFF = 4096
B = 2
S = 2048
NCORES = 8
GRP = 4
TOK = 512
EPS = 1e-5

AGH = 128 * 8 * 256      # elems contributed per core per AG half (bf16)
A2A_N = NCORES * 128 * TOK


def build_nc():
    nc = bacc.Bacc("TRN2", target_bir_lowering=False, debug=False,
                   num_devices=NCORES)

    x_own = nc.dram_tensor("x_own", [TOK, D], F32, kind="ExternalInput").ap()
    wq = nc.dram_tensor("wq", [128, 8, B, 128], mybir.dt.float8e4,
                        kind="ExternalInput").ap()
    wk = nc.dram_tensor("wk", [128, 8, B, 128], mybir.dt.float8e4,
                        kind="ExternalInput").ap()
    wv = nc.dram_tensor("wv", [128, 8, B, 128], mybir.dt.float8e4,
                        kind="ExternalInput").ap()
    bq = nc.dram_tensor("bq", [B, 128], F32, kind="ExternalInput").ap()
    bk = nc.dram_tensor("bk", [B, 128], F32, kind="ExternalInput").ap()
    wo = nc.dram_tensor("wo", [128, 8, D], mybir.dt.float8e4,
                        kind="ExternalInput").ap()
    bo = nc.dram_tensor("bo", [D], F32, kind="ExternalInput").ap()
    w1 = nc.dram_tensor("w1", [128, 32, 8, 128], BF16, kind="ExternalInput").ap()
    b1 = nc.dram_tensor("b1", [DFF], F32, kind="ExternalInput").ap()
    w2 = nc.dram_tensor("w2", [128, 8, 4, 8, 128], BF16, kind="ExternalInput").ap()
    b2 = nc.dram_tensor("b2", [D], F32, kind="ExternalInput").ap()
    me = nc.dram_tensor("me", [128, 256], BF16, kind="ExternalInput").ap()
    mo = nc.dram_tensor("mo", [128, 256], BF16, kind="ExternalInput").ap()
    id128 = nc.dram_tensor("id128", [128, 128], BF16, kind="ExternalInput").ap()
    out = nc.dram_tensor("out", [TOK, D], F32, kind="ExternalOutput").ap()

    rg = [list(range(NCORES))]

    with tile.TileContext(nc) as tc:
        with (
            tc.tile_pool(name="persist", bufs=1) as pp,
            tc.tile_pool(name="stage", bufs=4) as stg,
            tc.tile_pool(name="stats", bufs=4) as stp,
            tc.tile_pool(name="dram", bufs=1, space="DRAM") as dp,
        ):
            # ---- persistent SBUF (weights on ACT queue; x on SP queue) ----
            eps_sb = pp.tile([128, 1], F32, name="eps_sb")
            nc.vector.memset(eps_sb, EPS)
            id_sb = pp.tile([128, 128], BF16, name="id_sb")
            nc.scalar.dma_start(out=id_sb, in_=id128)
            me_sb = pp.tile([128, 256], BF16, name="me_sb")
            nc.scalar.dma_start(out=me_sb, in_=me)
            mo_sb = pp.tile([128, 256], BF16, name="mo_sb")
            nc.scalar.dma_start(out=mo_sb, in_=mo)
            bo_sb = pp.tile([128, 8], F32, name="bo_sb")
            nc.scalar.dma_start(out=bo_sb, in_=bo.rearrange("(k p) -> p k", p=128))
            b1_sb = pp.tile([128, 32], F32, name="b1_sb")
            nc.scalar.dma_start(out=b1_sb, in_=b1.rearrange("(k p) -> p k", p=128))
            b2_sb = pp.tile([128, 8], F32, name="b2_sb")
            nc.scalar.dma_start(out=b2_sb, in_=b2.rearrange("(k p) -> p k", p=128))

            x_tok = pp.tile([128, 4, D], F32, name="x_tok")
            x4 = x_own.rearrange("(t p) d -> t p d", p=128)
            for t in range(4):
                nc.sync.dma_start(out=x_tok[:, t, :], in_=x4[t])

            wq_sb = pp.tile([128, 8, B, 128], mybir.dt.float8e4,
                           name="wq_sb")
            nc.scalar.dma_start(out=wq_sb, in_=wq)
            wk_sb = pp.tile([128, 8, B, 128], mybir.dt.float8e4,
                           name="wk_sb")
            nc.scalar.dma_start(out=wk_sb, in_=wk)
            wv_sb = pp.tile([128, 8, B, 128], mybir.dt.float8e4,
                           name="wv_sb")
            nc.scalar.dma_start(out=wv_sb, in_=wv)
            bq_sb = pp.tile([128, B], F32, name="bq_sb")
            nc.scalar.dma_start(out=bq_sb, in_=bq.rearrange("b p -> p b"))
            bk_sb = pp.tile([128, B], F32, name="bk_sb")
            nc.scalar.dma_start(out=bk_sb, in_=bk.rearrange("b p -> p b"))

            qT = pp.tile([128, B, S], BF16, name="qT")
            kT = pp.tile([128, B, S], BF16, name="kT")
            vtok = pp.tile([128, 32, 2, 128], BF16, name="vtok")
            nc.vector.memset(vtok[:, :, :, 64:128], 1.0)
            attnT = pp.tile([128, B, S], BF16, name="attnT")
            attn8 = pp.tile([128, B, S], mybir.dt.float8e4, name="attn8")
            lnT = pp.tile([128, 8, TOK], BF16, name="lnT")
            lnT1 = pp.tile([128, 8, TOK], mybir.dt.float8e4, name="lnT1")

            # ---- DRAM ----
            ag_in = [dp.tile([AGH], mybir.dt.float8e4, name=f"ag_in{h}")
                     for h in range(2)]
            ag_out = [dp.tile([NCORES * AGH], mybir.dt.float8e4,
                              name=f"ag_out{h}", addr_space="Shared")
                      for h in range(2)]
            a2a_in = dp.tile([A2A_N], mybir.dt.float8e4, name="a2a_in")
            a2a_out = dp.tile([A2A_N], mybir.dt.float8e4, name="a2a_out")

            # ---- LN helper (token-major stats; bf16 out) ----
            def layernorm_chunk(src, t, xout):
                # src [128, 4, 1024] f32; writes xout [128, 1024] bf16
                xin = src[:, t, :]
                xg_ = xin.rearrange("p (g d) -> p g d", g=2)
                st_ = stp.tile([128, 2, 6], F32, name="st_")
                for gs in range(2):
                    nc.vector.bn_stats(out=st_[:, gs, :], in_=xg_[:, gs, :])
                mv = stp.tile([128, 2], F32, name="mv")
                nc.vector.bn_aggr(out=mv, in_=st_)
                rstd = stp.tile([128, 1], F32, name="rstd")
                nc.scalar.activation(out=rstd, in_=mv[:, 1:2], func=AF.Sqrt,
                                     bias=eps_sb, scale=1.0)
                nc.vector.reciprocal(out=rstd, in_=rstd)
                nc.vector.tensor_scalar(out=xout, in0=xin, scalar1=mv[:, 0:1],
                                        scalar2=rstd,
                                        op0=mybir.AluOpType.subtract,
                                        op1=mybir.AluOpType.mult)

            with (
                tc.tile_pool(name="xgp", bufs=1) as xgp,
                tc.tile_pool(name="psT1", bufs=2, space="PSUM") as psT1,
                tc.tile_pool(name="psA", bufs=3, space="PSUM") as psA,
                tc.tile_pool(name="psV", bufs=3, space="PSUM") as psV,
            ):
                def ln_transpose(src, t):
                    # LN of token chunk t -> feature-major lnT1 (fp8 cast in
                    # the PSUM->SBUF copy)
                    xln = stg.tile([128, D], BF16, name="xln")
                    layernorm_chunk(src, t, xln)
                    for fb in range(8):
                        pt = psT1.tile([128, 128], BF16, name="pt")
                        nc.tensor.transpose(pt, xln[:, fb * 128:(fb + 1) * 128],
                                            id_sb)
                        nc.vector.tensor_copy(
                            out=lnT1[:, fb, t * 128:(t + 1) * 128], in_=pt)

                # ---- LN1 + staged AllGather (2 token-halves, fp8) ----
                for h in range(2):
                    for tt in range(2):
                        ln_transpose(x_tok, 2 * h + tt)
                    nc.sync.dma_start(
                        out=ag_in[h].rearrange("(p k t) -> p k t", p=128, k=8),
                        in_=lnT1[:, :, h * 256:(h + 1) * 256])
                    nc.gpsimd.collective_compute(
                        "AllGather", mybir.AluOpType.bypass, replica_groups=rg,
                        ins=[ag_in[h].opt()], outs=[ag_out[h].opt()])

                # xg: [128 feat, src, half, kc, 256 tok] bf16
                xg = xgp.tile([128, NCORES, 2, 8, 256], mybir.dt.float8e4,
                              name="xg")
                for h in range(2):
                    ago = ag_out[h].rearrange("(r p k t) -> r p k t",
                                              r=NCORES, p=128, k=8)
                    for r in range(NCORES):
                        eng = [nc.sync, nc.scalar][r % 2]
                        eng.dma_start(out=xg[:, r, h], in_=ago[r])

                # ---- QKV per (half, src): Q,K feature-major; V token-major ----
                for h in range(2):
                    for r in range(NCORES):
                        b = r // 4
                        c0 = (r % 4) * 512 + h * 256
                        for dst, wsb, bsb in ((qT, wq_sb, bq_sb),
                                              (kT, wk_sb, bk_sb)):
                            acc = psA.tile([128, 256], F32, name="acc")
                            for kc in range(8):
                                nc.tensor.matmul(acc, lhsT=wsb[:, kc, b, :],
                                                 rhs=xg[:, r, h, kc, :],
                                                 start=(kc == 0), stop=(kc == 7))
                            nc.vector.tensor_scalar(
                                out=dst[:, b, c0:c0 + 256], in0=acc,
                                scalar1=1.0 / 64.0,
                                scalar2=bsb[:, b:b + 1],
                                op0=mybir.AluOpType.mult,
                                op1=mybir.AluOpType.add)
                        for tc_ in range(2):
                            accv = psV.tile([128, 128], F32, name="accv")
                            for kc in range(8):
                                nc.tensor.matmul(
                                    accv,
                                    lhsT=xg[:, r, h, kc,
                                            tc_ * 128:tc_ * 128 + 128],
                                    rhs=wv_sb[:, kc, b, :],
                                    start=(kc == 0), stop=(kc == 7))
                            ch = b * 16 + (r % 4) * 4 + h * 2 + tc_
                            nc.vector.tensor_scalar_mul(
                                out=vtok[:, ch, :, 0:64],
                                in0=accv.rearrange("p (hd c) -> p hd c", hd=2),
                                scalar1=1.0 / 64.0)

            # ---- attention: kt-major, 2-head row-packed scores ----
            with tc.tile_pool(name="ffp", bufs=1) as ffp:
                af_sb = ffp.tile([128, 8, TOK], mybir.dt.float8e4,
                                 name="af_sb")
                wo_sb = ffp.tile([128, 8, D], mybir.dt.float8e4,
                                 name="wo_sb")
                nc.scalar.dma_start(out=wo_sb, in_=wo)
                h1T = ffp.tile([128, 32, TOK], BF16, name="h1T")

                with (
                    tc.tile_pool(name="ptp", bufs=4) as ptp,
                    tc.tile_pool(name="rp", bufs=6) as rp,
                    tc.tile_pool(name="psS", bufs=2, space="PSUM") as psS,
                    tc.tile_pool(name="psO", bufs=4, space="PSUM") as psO,
                ):
                    def emit_pv(b, q2lo_h, q2hi, Ot, kt, q2lo, pt_):
                        for q2 in range(q2lo, q2hi):
                            qr = (q2 - q2lo) * 256
                            for hh in range(2):
                                # start=True clears has_written bits for the
                                # WHOLE bank; two q2 share one bank, so only
                                # the bank-first q2 may clear. The sibling's
                                # first write lands on cleared bits and
                                # overwrites correctly.
                                nc.tensor.matmul(
                                    Ot[hh][:, q2 - q2lo_h, :],
                                    lhsT=vtok[:, b * 16 + kt, hh, :],
                                    rhs=pt_[:, hh, qr:qr + 256],
                                    start=(kt == 0
                                           and (q2 - q2lo_h) % 2 == 0),
                                    stop=(kt == 2 * q2 + 1),
                                    skip_group_check=True)

                    # quarter-rounds: (b, qq) covers q2 pair (2qq, 2qq+1)
                    for b in range(B):
                        for qq in range(4):
                            q2lo_h, q2hi = 2 * qq, 2 * qq + 2
                            nkt = 4 * qq + 4
                            # O (per head): [128, 2 q2, 256]; rows 64:127 get
                            # the denominator replicated via vtok's ones cols
                            Ot = [psO.tile([128, 2, 256], F32, name="O")
                                  for _ in range(2)]
                            pend = None
                            for kt in range(nkt):
                                q2lo = max(q2lo_h, kt // 2)
                                ncols = (q2hi - q2lo) * 256
                                sc = psS.tile([128, 2, 512], F32, name="sc")
                                for hh in range(2):
                                    hp = hh * 64
                                    nc.tensor.matmul(
                                        sc[:, hh, :ncols],
                                        lhsT=kT[hp:hp + 64, b,
                                                kt * 128:kt * 128 + 128],
                                        rhs=qT[hp:hp + 64, b,
                                               q2lo * 256:q2lo * 256 + ncols],
                                        start=True, stop=True)
                                pt_ = ptp.tile([128, 2, 512], BF16,
                                               name="pt_")
                                nc.scalar.activation(out=pt_[:, :, :ncols],
                                                     in_=sc[:, :, :ncols],
                                                     func=AF.Exp)
                                # causal mask on the diagonal 256-block
                                if kt // 2 >= q2lo_h:
                                    msk = me_sb if kt % 2 == 0 else mo_sb
                                    for hh in range(2):
                                        nc.vector.tensor_mul(
                                            out=pt_[:, hh, 0:256],
                                            in0=pt_[:, hh, 0:256], in1=msk)
                                # software pipeline: PV of previous kt issues
                                # after this kt's score matmuls
                                if pend is not None:
                                    emit_pv(b, q2lo_h, q2hi, Ot, *pend)
                                pend = (kt, q2lo, pt_)
                            emit_pv(b, q2lo_h, q2hi, Ot, *pend)

                            # normalize: denominator rows are broadcast by the
                            # matmul; copy to SBUF (approx-recip can't read
                            # PSUM), reciprocal, fused multiply per head
                            for hh in range(2):
                                hp = hh * 64
                                for q2 in range(q2lo_h, q2hi):
                                    O2 = Ot[hh][:, q2 - q2lo_h, :]
                                    den = rp.tile([64, 256], F32, name="den")
                                    nc.vector.tensor_copy(out=den,
                                                          in_=O2[64:128, :])
                                    rc = rp.tile([64, 256], F32, name="rc")
                                    nc.vector.reciprocal_approx_fast(
                                        out=rc, in_=den)
                                    nc.vector.tensor_mul(
                                        out=attnT[hp:hp + 64, b,
                                                  q2 * 256:q2 * 256 + 256],
                                        in0=O2[0:64, :], in1=rc)
                            # fp8-cast this round's block (cast must be a
                            # tensor_copy for neuronxcc)
                            nc.vector.tensor_copy(
                                out=attn8[:, b, q2lo_h * 256:q2hi * 256],
                                in_=attnT[:, b, q2lo_h * 256:q2hi * 256])

                        # ship this batch's shards while the next computes
                        nc.sync.dma_start(
                            out=a2a_in.rearrange("(s p t) -> p s t", s=8,
                                                 p=128)[:, b * 4:(b + 1) * 4],
                            in_=attn8[:, b, :].rearrange("p (jj t) -> p jj t",
                                                          jj=4))

                nc.gpsimd.collective_compute(
                    "AllToAll", mybir.AluOpType.bypass, replica_groups=rg,
                    ins=[a2a_in.opt()], outs=[a2a_out.opt()])

                _af3 = a2a_out.rearrange("(i p t) -> i p t", i=8, p=128)
                for i in range(8):
                    [nc.gpsimd, nc.scalar][i % 2].dma_start(
                        out=af_sb[:, i, :], in_=_af3[i])

                # ---- wo projection + residual; LN2; FFN ----
                with (
                    tc.tile_pool(name="psW", bufs=3, space="PSUM") as psW,
                    tc.tile_pool(name="psT2", bufs=2, space="PSUM") as psT2,
                    tc.tile_pool(name="w2p", bufs=6) as w2p,
                ):
                    def resid_add(ybf, fb):
                        # ybf [128 feat, 512 tok] bf16 -> x_tok += y^T
                        for t in range(4):
                            pt = psT2.tile([128, 128], BF16, name="pt")
                            nc.tensor.transpose(
                                pt, ybf[:, t * 128:(t + 1) * 128], id_sb)
                            nc.vector.tensor_add(
                                out=x_tok[:, t, fb * 128:(fb + 1) * 128],
                                in0=x_tok[:, t, fb * 128:(fb + 1) * 128],
                                in1=pt)

                    for fb in range(8):
                        acc = psW.tile([128, TOK], F32, name="acc")
                        for kc in range(8):
                            nc.tensor.matmul(
                                acc, lhsT=wo_sb[:, kc, fb * 128:(fb + 1) * 128],
                                rhs=af_sb[:, kc, :],
                                start=(kc == 0), stop=(kc == 7))
                        yT = stg.tile([128, TOK], BF16, name="yT")
                        nc.vector.tensor_scalar(
                            out=yT, in0=acc, scalar1=1.0 / 64.0,
                            scalar2=bo_sb[:, fb:fb + 1],
                            op0=mybir.AluOpType.mult,
                            op1=mybir.AluOpType.add)
                        resid_add(yT, fb)

                    # ---- LN2 -> lnT (PE transposes) ----
                    for t in range(4):
                        xln = stg.tile([128, D], BF16, name="xln")
                        layernorm_chunk(x_tok, t, xln)
                        for fb in range(8):
                            pt = psT2.tile([128, 128], BF16, name="pt")
                            nc.tensor.transpose(
                                pt, xln[:, fb * 128:(fb + 1) * 128], id_sb)
                            nc.vector.tensor_copy(
                                out=lnT[:, fb, t * 128:(t + 1) * 128], in_=pt)

                    # ---- FFN1 ----
                    with tc.tile_pool(name="w1p", bufs=6) as w1p:
                        for hbk in range(32):
                            w1t = w1p.tile([128, 8, 128], BF16, name="w1t")
                            nc.sync.dma_start(out=w1t, in_=w1[:, hbk])
                            acc = psW.tile([128, TOK], F32, name="acc")
                            for kc in range(8):
                                nc.tensor.matmul(acc, lhsT=w1t[:, kc, :],
                                                 rhs=lnT[:, kc, :],
                                                 start=(kc == 0),
                                                 stop=(kc == 7))
                            nc.scalar.activation(out=h1T[:, hbk, :], in_=acc,
                                                 func=AF.Gelu,
                                                 bias=b1_sb[:, hbk:hbk + 1],
                                                 scale=1.0)

                    # ---- FFN2 + residual + streamed output ----
                    for fb in range(8):
                        acc = psW.tile([128, TOK], F32, name="acc")
                        for hg in range(4):
                            w2t = w2p.tile([128, 8, 128], BF16, name="w2t")
                            nc.sync.dma_start(out=w2t, in_=w2[:, fb, hg])
                            for kc in range(8):
                                nc.tensor.matmul(acc, lhsT=w2t[:, kc, :],
                                                 rhs=h1T[:, hg * 8 + kc, :],
                                                 start=(hg == 0 and kc == 0),
                                                 stop=(hg == 3 and kc == 7))
                        y2T = stg.tile([128, TOK], BF16, name="y2T")
                        nc.vector.tensor_scalar_add(out=y2T, in0=acc,
                                                    scalar1=b2_sb[:, fb:fb + 1])
                        resid_add(y2T, fb)
                        ov = out.rearrange("(t p) d -> p t d", p=128)
                        for t in range(4):
                            [nc.gpsimd, nc.scalar][t % 2].dma_start(
                                out=ov[:, t, fb * 128:(fb + 1) * 128],
                                in_=x_tok[:, t, fb * 128:(fb + 1) * 128])

    nc.compile()
    return nc


_NC_CACHE = {}


def _get_nc():
    if "nc" not in _NC_CACHE:
        _NC_CACHE["nc"] = build_nc()
    return _NC_CACHE["nc"]


def _prep_in_maps(x, ln1_g, ln1_b, wq, bq, wk, bk, wv, bv, wo, bo,
                  ln2_g, ln2_b, w1, b1, w2, b2):
    bf16 = ml_dtypes.bfloat16
    f32 = np.float32
    x = np.asarray(x, f32)
    DK = 64
    sc = 1.0 / np.sqrt(DK)
    ln1_g = np.asarray(ln1_g, f32)
    ln1_b = np.asarray(ln1_b, f32)
    ln2_g = np.asarray(ln2_g, f32)
    ln2_b = np.asarray(ln2_b, f32)
    wq = np.asarray(wq, f32)
    wk = np.asarray(wk, f32)
    wv = np.asarray(wv, f32)
    wo_np = np.asarray(wo, f32)
    w1 = np.asarray(w1, f32)
    w2 = np.asarray(w2, f32)

    f8 = ml_dtypes.float8_e4m3
    wq_f = (ln1_g[:, None] * wq * sc * 64.0).astype(f8)
    bq_f = ((ln1_b @ wq + np.asarray(bq, f32)) * sc).astype(f32)
    wk_f = (ln1_g[:, None] * wk * 64.0).astype(f8)
    bk_f = (ln1_b @ wk + np.asarray(bk, f32)).astype(f32)
    wv_f = (ln1_g[:, None] * wv * 64.0).astype(f8)
    bv_f = (ln1_b @ wv + np.asarray(bv, f32)).astype(f32)
    bo_f = (np.asarray(bo, f32) + bv_f @ wo_np).astype(f32)
    wo_f = (wo_np * 64.0).astype(ml_dtypes.float8_e4m3)
    w1_f = (ln2_g[:, None] * w1).astype(bf16)
    b1_f = (ln2_b @ w1 + np.asarray(b1, f32)).astype(f32)
    w2_f = w2.astype(bf16)
    b2_f = np.asarray(b2, f32)

    tri = np.triu(np.ones((128, 128), f32))
    me_np = np.concatenate([tri, np.ones((128, 128), f32)], 1).astype(bf16)
    mo_np = np.concatenate([np.zeros((128, 128), f32), tri], 1).astype(bf16)
    id_np = np.eye(128, dtype=f32).astype(bf16)

    in_maps = []
    for core in range(NCORES):
        g, l = divmod(core, GRP)

        def hsel(b, j=core):
            m = (j + 4 * b) % 8
            return slice(m * 128, (m + 1) * 128)

        wo_perm = np.concatenate(
            [wo_f[((i + 4 * g) % 8) * 128:((i + 4 * g) % 8) * 128 + 128, :]
             for i in range(8)], axis=0)

        def pmaj(a):   # [1024, ...] -> [128, 8, ...] partition-major
            return np.ascontiguousarray(
                a.reshape(8, 128, *a.shape[1:]).transpose(
                    1, 0, *range(2, a.ndim + 1)))

        in_maps.append({
            "x_own": np.ascontiguousarray(x[g, l * TOK:(l + 1) * TOK, :]),
            "wq": pmaj(np.stack([wq_f[:, hsel(b)] for b in range(B)], axis=1)),
            "wk": pmaj(np.stack([wk_f[:, hsel(b)] for b in range(B)], axis=1)),
            "wv": pmaj(np.stack([wv_f[:, hsel(b)] for b in range(B)], axis=1)),
            "bq": np.ascontiguousarray(
                np.stack([bq_f[hsel(b)] for b in range(B)])),
            "bk": np.ascontiguousarray(
                np.stack([bk_f[hsel(b)] for b in range(B)])),
            "wo": pmaj(wo_perm), "bo": bo_f,
            "w1": np.ascontiguousarray(
                w1_f.reshape(8, 128, 32, 128).transpose(1, 2, 0, 3)),
            "b1": b1_f,
            "w2": np.ascontiguousarray(
                w2_f.reshape(4, 8, 128, 8, 128).transpose(2, 3, 0, 1, 4)),
            "b2": b2_f,
            "me": me_np, "mo": mo_np, "id128": id_np,
        })
    return in_maps


def kernel(**inputs):
    nc = _get_nc()
    in_maps = _prep_in_maps(**inputs)
    res = run_bass_kernel_spmd(nc, in_maps, core_ids=list(range(NCORES)))
    full = np.empty((B, S, D), np.float32)
    for core in range(NCORES):
        g, l = divmod(core, GRP)
        full[g, l * TOK:(l + 1) * TOK, :] = res.results[core]["out"]
    return full
